# revision 2
# baseline (speedup 1.0000x reference)
"""Trainium2 Bass kernel v2 for nn_DensePoseV1ConvXGNInsHead:
2x (conv3x3 64->64 -> per-instance BN -> ReLU) on [8,64,256,256],
one image per NeuronCore across 8 cores.

v2 redesign vs baseline:
- conv matmuls use full 128-partition block-diagonal weights (one MM per
  tap for both row-halves) instead of 64x64 quadrant pairs
- stats finalize swaps partition halves with a PE permutation matmul
  (no serial SBUF->SBUF DMAs on the critical path)
- all masks host-prepared and DMA'd contiguously
- L0-normalize and L1-conv interleaved in emission order so PE never drains
- f16 output
"""
import os
import sys
import types

sys.path.insert(0, "/opt/trn_rl_repo")

import numpy as np

import concourse.bass as bass
import concourse.tile as tile
from concourse import mybir
from concourse.vector_clock import ScopedClock

f16 = mybir.dt.float16
f32 = mybir.dt.float32
ALU = mybir.AluOpType
ACTF = mybir.ActivationFunctionType

C = 64          # channels
W = 256         # image width
PITCH = 272     # padded row pitch (16 left pad + 256 data; borrows next row's pad)
LP = 16         # left pad elements
EPS = 1e-5

# ---------------------------------------------------------------------------
# walrus workaround: split the Tile exit-drain's sem waits (installed walrus
# rejects instructions with >2 sync waits)
# ---------------------------------------------------------------------------
_patched = False


def _install_tile_patch():
    global _patched
    if _patched:
        return
    _patched = True

    def _drain_and_barrier(self, tick_clock, wait_clock):
        nc = self.nc
        drain_inst = nc.sync.drain()
        wait_clock.add_sem_waits(
            drain_inst.ins, ScopedClock({None: tick_clock.global_clock})
        )
        si = drain_inst.ins.sync_info
        waits = list(si.on_wait or [])
        if len(waits) > 1:
            si.on_wait = waits[:1]
            for i in range(1, len(waits)):
                nop = nc.sync.nop()
                nop.ins.sync_info = mybir.SyncInfo(
                    on_wait=waits[i : i + 1], on_update=[]
                )
        nc.all_engine_barrier()
        popped = nc._tile_sem_poison_stack.pop()
        assert popped is self._sem_poison
        nc.clear_and_free_semaphores(list(self.sems.allocated().values()))
        nc.all_engine_barrier()

    tile.TileContext._drain_and_barrier = _drain_and_barrier


# ---------------------------------------------------------------------------
# NTFF profiling shim (antenv.axon_hooks is absent in this image)
# ---------------------------------------------------------------------------
def _install_ntff_shim():
    if "antenv.axon_hooks" in sys.modules:
        return
    mod = types.ModuleType("antenv.axon_hooks")
    state = {"hook": None}
    mod.set_axon_ntff_profile_hook = lambda h: state.__setitem__("hook", h)
    mod.get_axon_ntff_profile_hook = lambda: state["hook"]
    sys.modules["antenv.axon_hooks"] = mod
    try:
        import antenv

        antenv.axon_hooks = mod
    except ImportError:
        pass
    try:
        from trn_agent_boot.trn_boot import _ntff_profile_via_ctypes

        h = _ntff_profile_via_ctypes("/opt/axon/libaxon_pjrt.so")
        mod.set_axon_ntff_profile_hook(h)
    except Exception:
        pass


def yoff(slot):
    return slot * PITCH + LP


def _ap(base_ap, offset_elems, dims):
    """Sub-AP of base_ap at +offset (elements) with given free dims."""
    return bass.AP(
        tensor=base_ap.tensor,
        offset=base_ap.offset + offset_elems,
        ap=[base_ap.ap[0]] + dims,
    )


def emit(nc, H=256):
    HH = H // 2          # 128 rows per half
    NCH = HH // 2        # conv chunks (2 rows) per layer = 64
    NPC = NCH * 4        # pixel chunks (128 px) per layer = 256
    NB = HH // 4         # x-load blocks (4 rows) = 32
    HW2 = HH * W

    xh = nc.declare_dram_parameter("xh", [C, (H + 1) * PITCH], f16, isOutput=False)
    w0d = nc.declare_dram_parameter("w0d", [128, 9, 128], f16, isOutput=False)
    w1d = nc.declare_dram_parameter("w1d", [128, 9, 128], f16, isOutput=False)
    id128 = nc.declare_dram_parameter("id128", [128, 128], f16, isOutput=False)
    idp_d = nc.declare_dram_parameter("idp", [128, 2 * NPC], f16, isOutput=False)
    krow_d = nc.declare_dram_parameter("krow", [128, 9], f16, isOutput=False)
    idsf_d = nc.declare_dram_parameter("idsf", [2, HW2], f16, isOutput=False)
    kv41_d = nc.declare_dram_parameter("kv41", [41], f32, isOutput=False)
    perm_d = nc.declare_dram_parameter("perm", [41, 41], f32, isOutput=False)
    rc_d = nc.declare_dram_parameter("rc", [41], f32, isOutput=False)
    g0d = nc.declare_dram_parameter("g0", [41, 64], f32, isOutput=False)
    b0d = nc.declare_dram_parameter("b0", [41, 64], f32, isOutput=False)
    g1d = nc.declare_dram_parameter("g1", [41, 64], f32, isOutput=False)
    b1d = nc.declare_dram_parameter("b1", [41, 64], f32, isOutput=False)
    out = nc.declare_dram_parameter("out", [C, H * W], f16, isOutput=True)

    with tile.TileContext(nc) as tc:
        import contextlib

        with contextlib.ExitStack() as ctx:
            const = ctx.enter_context(tc.tile_pool(name="const", bufs=1))
            xbp = ctx.enter_context(tc.tile_pool(name="xbp", bufs=1))
            spp = ctx.enter_context(tc.tile_pool(name="spp", bufs=4))
            normp = ctx.enter_context(tc.tile_pool(name="normp", bufs=3))
            outp = ctx.enter_context(tc.tile_pool(name="outp", bufs=4))
            smallp = ctx.enter_context(tc.tile_pool(name="smallp", bufs=2))
            psc = ctx.enter_context(tc.tile_pool(name="psc", bufs=2, space="PSUM"))
            ptp = ctx.enter_context(tc.tile_pool(name="ptp", bufs=2, space="PSUM"))
            pss = ctx.enter_context(tc.tile_pool(name="pss", bufs=1, space="PSUM"))
            pse = ctx.enter_context(tc.tile_pool(name="pse", bufs=2, space="PSUM"))
            psw = ctx.enter_context(tc.tile_pool(name="psw", bufs=1, space="PSUM"))

            # ---- persistent y buffer (both layers, in-place)
            ysb = const.tile([128, (HH + 2) * PITCH + LP], f16)
            # zero only what stays zero: inter-row pads + top/bottom halo slots
            nc.vector.memset(_ap(ysb[:], 0, [[PITCH, HH + 3], [1, LP]]), 0.0)
            nc.vector.memset(_ap(ysb[:], yoff(0), [[1, W]]), 0.0)
            nc.vector.memset(_ap(ysb[:], yoff(HH + 1), [[1, W]]), 0.0)
            xb0 = xbp.tile([128, 6 * PITCH + LP], f16, tag="xb0")
            xb1 = xbp.tile([128, 6 * PITCH + LP], f16, tag="xb1")
            nc.vector.memset(xb0[:], 0.0)
            nc.vector.memset(xb1[:], 0.0)
            xbs = [xb0, xb1]

            # ---- constants from DRAM; only weights + identity loaded NOW.
            # Masks and finalize constants are DMA'd later (emission order =
            # queue order: they must not delay the first x loads).
            wts = []
            for wd in (w0d, w1d):
                wt = const.tile([128, 9, 128], f16, tag="wt", name=f"wt{len(wts)}")
                nc.sync.dma_start(out=wt[:], in_=wd[:])
                wts.append(wt)
            id128sb = const.tile([128, 128], f16)
            nc.sync.dma_start(out=id128sb[:], in_=id128[:])
            idp = const.tile([128, 2 * NPC], f16)
            nc.sync.dma_start(out=idp[:], in_=idp_d[:])
            krow = const.tile([128, 9], f16)
            nc.sync.dma_start(out=krow[:], in_=krow_d[:])
            # stats mask generated on-chip (saves 2.7MB of DMA):
            # cols 0:9 = (idA==k), 32:41 = (idB==k), 9:32 zero
            mpm = const.tile([128, NPC, 41], f16)
            nc.vector.memset(_ap(mpm[:], 9, [[41, NPC], [1, 23]]), 0.0)
            nc.vector.tensor_tensor(
                _ap(mpm[:], 0, [[41, NPC], [1, 9]]),
                _ap(idp[:], 0, [[1, NPC], [0, 9]]),
                _ap(krow[:], 0, [[0, NPC], [1, 9]]),
                ALU.is_equal,
            )
            nc.vector.tensor_tensor(
                _ap(mpm[:], 32, [[41, NPC], [1, 9]]),
                _ap(idp[:], NPC, [[1, NPC], [0, 9]]),
                _ap(krow[:], 0, [[0, NPC], [1, 9]]),
                ALU.is_equal,
            )
            ms2sb = const.tile([41, HW2], f16)
            kv41 = const.tile([41, 1], f32)
            nc.scalar.dma_start(
                out=kv41[:], in_=kv41_d[:].rearrange("(a b) -> a b", b=1))
            MSL = HW2 // 16

            def gen_ms2_slice(k):
                """Fill ms2sb[:, k*MSL:(k+1)*MSL] from broadcast ids.

                Rows 0:9 <- (idA==s), rows 32:41 <- (idB==s); rows 9:32
                compare stale data against NaN -> always 0."""
                ids41 = spp.tile([41, MSL], f16, tag="ids41", name=f"ids41_{k}",
                                 bufs=2)
                nc.sync.dma_start(
                    out=ids41[0:9, :],
                    in_=bass.AP(tensor=idsf_d[:].tensor, offset=k * MSL,
                                ap=[[0, 9], [1, MSL]]),
                )
                nc.scalar.dma_start(
                    out=ids41[32:41, :],
                    in_=bass.AP(tensor=idsf_d[:].tensor, offset=HW2 + k * MSL,
                                ap=[[0, 9], [1, MSL]]),
                )
                nc.vector.tensor_scalar(
                    out=ms2sb[:, k * MSL : (k + 1) * MSL], in0=ids41[:],
                    scalar1=kv41[:], scalar2=None, op0=ALU.is_equal,
                )
            permsb = const.tile([41, 41], f32)
            rcsb = const.tile([41, 1], f32)
            gbt = []
            for li in (0, 1):
                gt = const.tile([41, 64], f32, tag="g18", name=f"g18_{li}")
                bt = const.tile([41, 64], f32, tag="b18", name=f"b18_{li}")
                gbt.append((gt, bt))
            epsap = const.tile([41, 1], f32)
            nc.vector.memset(epsap[:], EPS)

            def load_late_consts_2():
                # finalize constants (scalar DGE queue; needed ~180us in)
                nc.scalar.dma_start(out=permsb[:], in_=perm_d[:])
                nc.scalar.dma_start(
                    out=rcsb[:], in_=rc_d[:].rearrange("(a b) -> a b", b=1))
                for li, (gg, bb) in enumerate(((g0d, b0d), (g1d, b1d))):
                    nc.scalar.dma_start(out=gbt[li][0][:], in_=gg[:])
                    nc.scalar.dma_start(out=gbt[li][1][:], in_=bb[:])



            # ---- per-layer affine tables [18, 128] f16 (block structure):
            # rows 0:9 hold A-seg values in cols 0:64, rows 9:18 B-seg in 64:128.
            # Background rows 8 / 17: scale 1, offset 0 (set once, never touched).
            ones16 = const.tile([1, 64], f16)
            nc.vector.memset(ones16[:], 1.0)
            abts = []
            for L in (0, 1):
                s_t = const.tile([41, 128], f16, tag="abs", name=f"ab2s{L}")
                o_t = const.tile([41, 128], f16, tag="abo", name=f"ab2o{L}")
                nc.vector.memset(s_t[:], 0.0)
                nc.vector.memset(o_t[:], 0.0)
                # background rows (seg 8): scale 1, offset 0
                nc.scalar.dma_start(out=s_t[8:9, 0:64], in_=ones16[:])
                nc.scalar.dma_start(out=s_t[40:41, 64:128], in_=ones16[:])
                abts.append((s_t, o_t))

            # =============================================================
            # emission helpers
            # =============================================================
            def load_x_block(b):
                """Load x rows for 4-row block b into xb[b%2] (rows r0-1..r0+4).

                xh is PITCH-padded in DRAM, so each half loads as ONE fully
                contiguous run per channel (3.3KB packets, pads included)."""
                xb = xbs[b % 2]
                r0 = 4 * b
                XP = (H + 1) * PITCH

                def half_load(part_lo, part_hi, xb_off, row_lo, n_elems, coff):
                    eng = nc.sync if part_lo == 0 else nc.scalar
                    eng.dma_start(
                        out=_ap(xb[part_lo:part_hi, :], xb_off, [[1, n_elems]]),
                        in_=bass.AP(
                            tensor=xh[:].tensor,
                            offset=coff + row_lo * PITCH,
                            ap=[[XP, 64], [1, n_elems]],
                        ),
                    )

                if b == 0:
                    nc.vector.memset(xb[0:64, 0:PITCH], 0.0)
                    # A rows 0..4 -> slots 1..5 (incl. borrow tail)
                    half_load(0, 64, PITCH, 0, 5 * PITCH + LP, 0)
                    # B rows HH-1..HH+4 -> slots 0..5
                    half_load(64, 128, 0, HH - 1, 6 * PITCH + LP, 0)
                elif b == NB - 1:
                    # A rows r0-1..r0+4 (row r0+4 == HH exists)
                    half_load(0, 64, 0, r0 - 1, 6 * PITCH + LP, 0)
                    nc.vector.memset(
                        xb[64:128, 5 * PITCH : 6 * PITCH + LP], 0.0)
                    # B rows HH+r0-1..HH+r0+3 -> slots 0..4
                    half_load(64, 128, 0, HH + r0 - 1, 5 * PITCH + LP, 0)
                else:
                    half_load(0, 64, 0, r0 - 1, 6 * PITCH + LP, 0)
                    half_load(64, 128, 0, HH + r0 - 1, 6 * PITCH + LP, 0)

            def conv_chunk(L, c):
                """9-tap conv for 2-row chunk c (both halves) + copy to ysb."""
                wt = wts[L]
                slot0 = 1 if L == 0 else 0
                rr = 2 * c
                pts = psc.tile([128, 512], f32, tag="cps", name=f"cps_{L}_{c}")
                if L == 0:
                    src_t = xbs[(c // 2) % 2]
                    base_slot = 2 * (c % 2) + 1  # row rr at block slot
                else:
                    src_t = ysb
                    base_slot = rr + 1
                for t in range(9):
                    dy, dx = t // 3 - 1, t % 3 - 1
                    off = yoff(base_slot + dy) + dx
                    rhs = _ap(src_t[:], off, [[PITCH, 2], [1, W]])
                    nc.tensor.matmul(
                        pts[:], _ap(wt[:], t * 128, [[1, 128]]), rhs,
                        start=(t == 0), stop=(t == 8),
                    )
                dst = _ap(ysb[:], yoff(rr + slot0), [[PITCH, 2], [1, W]])
                nc.vector.tensor_copy(dst, pts[:])

            def stats_transpose(L, c):
                """PE-transpose the 4 128-px chunks of conv chunk c."""
                slot0 = 1 if L == 0 else 0
                pts2 = ptp.tile([128, 512], f16, tag="tp", name=f"tp_{L}_{c}")
                for j in range(4):
                    src = _ap(
                        ysb[:], yoff(2 * c + j // 2 + slot0) + (j % 2) * 128,
                        [[1, 128]],
                    )
                    nc.tensor.transpose(
                        pts2[:, j * 128 : (j + 1) * 128], src, id128sb[:]
                    )
                return pts2

            def stats_strip(L, c, pts2):
                """Copy transposed y to SBUF and square it."""
                sp = spp.tile([128, 1024], f16, tag="strip", name=f"sp_{L}_{c}")
                nc.scalar.copy(out=sp[:, 0:512], in_=pts2[:])
                nc.vector.tensor_tensor(
                    sp[:, 512:1024], sp[:, 0:512], sp[:, 0:512], ALU.mult
                )
                return sp

            def stats_mm(L, c, sp, stats):
                for j in range(4):
                    ci = 4 * c + j
                    nc.tensor.matmul(
                        stats[:],
                        _ap(mpm[:], ci * 41, [[1, 41]]),
                        _ap(sp[:], j * 128, [[512, 2], [1, 128]]),
                        start=(ci == 0), stop=(ci == NPC - 1),
                    )

            def finalize(L, stats):
                """stats psum [41,256] -> affine tables ab2s/ab2o for layer L.

                Rows 0:9 = A-half segment sums, rows 32:41 = B-half (32-aligned
                partition bases for engine-op legality); rows 9:32 stay zero."""
                gt, bt = gbt[L]
                s_t, o_t = abts[L]
                ssb = smallp.tile([41, 256], f32, tag="ssb", name=f"ssb{L}")
                nc.scalar.copy(out=ssb[:], in_=stats[:])
                # PE permutation: swp[s] <-> ssb[32+s] (A/B half swap)
                swp = psw.tile([41, 256], f32, tag="swp", name=f"swp{L}")
                nc.tensor.matmul(swp[:], permsb[:], ssb[:], start=True, stop=True)
                s1 = smallp.tile([41, 64], f32, tag="s1", name=f"s1_{L}")
                s2 = smallp.tile([41, 64], f32, tag="s2", name=f"s2_{L}")
                nc.vector.memset(s1[:], 0.0)
                nc.vector.memset(s2[:], 0.0)
                # own half data in own cols; other half arrives swapped
                nc.vector.tensor_tensor(
                    s1[0:9, :], ssb[0:9, 0:64], swp[0:9, 64:128], ALU.add)
                nc.vector.tensor_tensor(
                    s1[32:41, :], ssb[32:41, 64:128], swp[32:41, 0:64], ALU.add)
                nc.vector.tensor_tensor(
                    s2[0:9, :], ssb[0:9, 128:192], swp[0:9, 192:256], ALU.add)
                nc.vector.tensor_tensor(
                    s2[32:41, :], ssb[32:41, 192:256], swp[32:41, 128:192], ALU.add)
                mean = smallp.tile([41, 64], f32, tag="mean", name=f"mean{L}")
                nc.vector.tensor_scalar_mul(out=mean[:], in0=s1[:], scalar1=rcsb[:])
                e2 = smallp.tile([41, 64], f32, tag="e2", name=f"e2_{L}")
                nc.vector.tensor_scalar_mul(out=e2[:], in0=s2[:], scalar1=rcsb[:])
                var = smallp.tile([41, 64], f32, tag="var", name=f"var{L}")
                nc.vector.tensor_tensor(var[:], mean[:], mean[:], ALU.mult)
                nc.vector.tensor_tensor(var[:], e2[:], var[:], ALU.subtract)
                sd = smallp.tile([41, 64], f32, tag="sd", name=f"sd{L}")
                nc.scalar.activation(
                    out=sd[:], in_=var[:], func=ACTF.Sqrt, bias=epsap[:], scale=1.0
                )
                rstd = smallp.tile([41, 64], f32, tag="rstd", name=f"rstd{L}")
                nc.vector.reciprocal(out=rstd[:], in_=sd[:])
                a = smallp.tile([41, 64], f32, tag="a", name=f"a{L}")
                nc.vector.tensor_tensor(a[:], rstd[:], gt[:], ALU.mult)
                am = smallp.tile([41, 64], f32, tag="am", name=f"am{L}")
                nc.vector.tensor_tensor(am[:], a[:], mean[:], ALU.mult)
                bv = smallp.tile([41, 64], f32, tag="bv", name=f"bv{L}")
                nc.vector.tensor_tensor(bv[:], bt[:], am[:], ALU.subtract)
                # quadrant placement (aligned bases; skip bg rows 8/40)
                nc.vector.tensor_copy(s_t[0:8, 0:64], a[0:8, :])
                nc.scalar.copy(out=s_t[32:40, 64:128], in_=a[32:40, :])
                nc.vector.tensor_copy(o_t[0:8, 0:64], bv[0:8, :])
                nc.scalar.copy(out=o_t[32:40, 64:128], in_=bv[32:40, :])

            def norm_group(L, g):
                """Normalize+ReLU rows 2g,2g+1 (both halves) of layer L.

                L=1 (tail phase, no conv to overlap): sE copy goes to DVE and
                oE lives in the psc pool (idle in phase C) for deeper overlap."""
                s_t, o_t = abts[L]
                slot0 = 1 if L == 0 else 0
                win = ms2sb[:, g * 512 : (g + 1) * 512]
                sE = pse.tile([128, 512], f32, tag="exp", name=f"se{L}_{g}")
                if L == 0:
                    oE = pse.tile([128, 512], f32, tag="exp", name=f"oe{L}_{g}")
                else:
                    oE = psc.tile([128, 512], f32, tag="cps", name=f"oe{L}_{g}")
                nc.tensor.matmul(sE[:], s_t[:], win, start=True, stop=True)
                nc.tensor.matmul(oE[:], o_t[:], win, start=True, stop=True)
                sEc = normp.tile([128, 512], f16, tag="sEc", name=f"sec{L}_{g}")
                oEc = normp.tile([128, 512], f16, tag="oEc", name=f"oec{L}_{g}")
                if L == 0:
                    nc.scalar.copy(out=sEc[:], in_=sE[:])
                else:
                    nc.vector.tensor_copy(sEc[:], sE[:])
                nc.scalar.copy(out=oEc[:], in_=oE[:])
                yv = _ap(ysb[:], yoff(2 * g + slot0), [[PITCH, 2], [1, W]])
                t1 = normp.tile([128, 512], f16, tag="t1", name=f"t1_{L}_{g}")
                u = normp.tile([128, 512], f16, tag="u", name=f"u_{L}_{g}")
                nc.vector.tensor_tensor(t1[:], yv, sEc[:], ALU.mult)
                nc.vector.tensor_tensor(u[:], t1[:], oEc[:], ALU.add)
                if L == 0:
                    dst = yv
                else:
                    st = outp.tile([128, 512], f16, tag="st", name=f"st{g}")
                    dst = st[:]
                if L == 0 and g % 2 == 0:
                    nc.vector.tensor_scalar_max(out=dst, in0=u[:], scalar1=0.0)
                else:
                    # L=1: DVE already carries the sE copy; relu stays on ACT
                    nc.scalar.activation(out=dst, in_=u[:], func=ACTF.Relu)
                if L == 1:
                    nc.sync.dma_start(
                        out=bass.AP(
                            tensor=out[:].tensor, offset=2 * g * W,
                            ap=[[HH * W, 2], [H * W, 64], [W, 2], [1, W]],
                        ),
                        in_=st[:],
                    )

            # =============================================================
            # phase A: layer-0 conv + stats (software-pipelined)
            # =============================================================
            stats0 = pss.tile([41, 256], f32, tag="stats", name="stats0")
            load_x_block(0)
            load_x_block(1)
            pend = {}
            for c in range(NCH):
                conv_chunk(0, c)
                if c == 2:
                    load_late_consts_2()
                if 6 <= c < 38 and c % 2 == 0:
                    gen_ms2_slice((c - 6) // 2)
                if c % 2 == 1 and c // 2 + 2 < NB:
                    # prefetch after block c//2's last consumer (WAR via emission order)
                    load_x_block(c // 2 + 2)
                if c >= 1:
                    p2 = stats_transpose(0, c - 1)
                    pend[c - 1] = stats_strip(0, c - 1, p2)
                if c >= 4:
                    stats_mm(0, c - 4, pend.pop(c - 4), stats0)
            p2 = stats_transpose(0, NCH - 1)
            pend[NCH - 1] = stats_strip(0, NCH - 1, p2)
            for cc in range(NCH - 4, NCH):
                stats_mm(0, cc, pend.pop(cc), stats0)

            finalize(0, stats0)

            # =============================================================
            # phase B: layer-0 normalize interleaved with layer-1 conv+stats
            # norm group order: [63, 0, 1, ..., 62] so halos are ready early
            # =============================================================
            stats1 = pss.tile([41, 256], f32, tag="stats", name="stats1")
            G = [NCH - 1] + list(range(NCH - 1))
            LAG = 4
            for i in range(NCH + LAG):
                if i < NCH:
                    norm_group(0, G[i])
                if i == 0:
                    # B-half bottom: B slot 0 <- A row HH-1 (slot HH)
                    nc.sync.dma_start(
                        out=_ap(ysb[64:128, :], yoff(0), [[1, W]]),
                        in_=_ap(ysb[0:64, :], yoff(HH), [[1, W]]),
                    )
                if i == 1:
                    # A-half top halo: A slot HH+1 <- B row 0 (slot 1)
                    nc.sync.dma_start(
                        out=_ap(ysb[0:64, :], yoff(HH + 1), [[1, W]]),
                        in_=_ap(ysb[64:128, :], yoff(1), [[1, W]]),
                    )
                if i >= LAG:
                    c = i - LAG
                    conv_chunk(1, c)
                    if c >= 1:
                        p2 = stats_transpose(1, c - 1)
                        pend[c - 1] = stats_strip(1, c - 1, p2)
                    if c >= 4:
                        stats_mm(1, c - 4, pend.pop(c - 4), stats1)
            p2 = stats_transpose(1, NCH - 1)
            pend[NCH - 1] = stats_strip(1, NCH - 1, p2)
            for cc in range(NCH - 4, NCH):
                stats_mm(1, cc, pend.pop(cc), stats1)

            finalize(1, stats1)

            # =============================================================
            # phase C: layer-1 normalize -> out
            # =============================================================
            for g in range(NCH):
                norm_group(1, g)

    return nc


MAXW = 1


def _split_multi_waits(nc):
    """The installed walrus rejects instructions with >MAXW sync waits; hoist
    excess waits onto preceding same-engine nops."""
    nsplit = 0
    for fn in nc.m.functions:
        for blk in fn.blocks:
            insts = list(blk.instructions)
            out = []
            for inst in insts:
                si = inst.sync_info
                waits = list(si.on_wait) if (si and si.on_wait) else []
                if len(waits) > MAXW:
                    for i in range(0, len(waits) - MAXW, MAXW):
                        nop = mybir.InstNoOp(
                            name=f"WSPLIT-{nsplit}", ins=[], outs=[]
                        )
                        nsplit += 1
                        nop.engine = inst.engine
                        nop.sync_info = mybir.SyncInfo(
                            on_wait=waits[i : i + MAXW], on_update=[]
                        )
                        out.append(nop)
                    si.on_wait = waits[len(waits) - MAXW :]
                out.append(inst)
            if len(out) != len(insts):
                while len(blk.instructions):
                    blk.instructions.pop()
                for inst in out:
                    blk.instructions.append(inst)
    return nsplit


def build_nc(H=256, split_waits=True):
    _install_tile_patch()
    nc = bass.Bass()
    emit(nc, H)
    if split_waits:
        n = _split_multi_waits(nc)
        if n:
            print(f"kernel: split {n} multi-wait instructions")
    return nc


# ---------------------------------------------------------------------------
# host-side input prep
# ---------------------------------------------------------------------------
def prep_core_inputs(x_img, ids_img, w0, g0v, b0v, w1, g1v, b1v, H=256):
    HH = H // 2
    NPC = HH * 2
    HW2 = HH * W
    seg = np.where(ids_img < 0, 8, ids_img).astype(np.int64)
    segA, segB = seg[:HH], seg[HH:]

    m = {}
    xpad = np.zeros((C, (H + 1) * PITCH), np.float16)
    xr = xpad.reshape(C, H + 1, PITCH)
    xr[:, :H, LP : LP + W] = x_img.astype(np.float16)
    m["xh"] = np.ascontiguousarray(xpad)
    cnt = np.bincount(seg.reshape(-1), minlength=9)[:9]
    rc9 = (1.0 / np.maximum(cnt, 1)).astype(np.float32)
    
    for name, wmat in (("w0d", w0), ("w1d", w1)):
        wd = np.zeros((9, 128, 128), np.float16)
        for t in range(9):
            dy, dx = t // 3, t % 3
            lhsT = wmat[:, :, dy, dx].T.astype(np.float16)  # [cin, cout]
            wd[t, 0:64, 0:64] = lhsT
            wd[t, 64:128, 64:128] = lhsT
        m[name] = np.ascontiguousarray(wd.transpose(1, 0, 2))  # [ci, t, co]

    m["id128"] = np.eye(128, dtype=np.float16)

    # pixel-major ids [128, 2, NPC]: [:,0,ci]=A-seg, [:,1,ci]=B-seg of the
    # pixel (r, cs*128+p), ci = 2*r + cs; stats mask built on-chip from this
    pA = segA.reshape(HH, 2, 128).transpose(2, 0, 1).reshape(128, NPC)
    pB = segB.reshape(HH, 2, 128).transpose(2, 0, 1).reshape(128, NPC)
    m["idp"] = np.ascontiguousarray(
        np.stack([pA, pB], 1).reshape(128, 2 * NPC).astype(np.float16))
    m["krow"] = np.ascontiguousarray(
        np.broadcast_to(np.arange(9, dtype=np.float16), (128, 9))).copy()

    # raw ids for on-chip expansion-mask build
    m["idsf"] = np.ascontiguousarray(
        np.stack([segA.reshape(-1), segB.reshape(-1)]).astype(np.float16))
    kv = np.full(41, np.nan, np.float32)
    kv[0:9] = np.arange(9)
    kv[32:41] = np.arange(9)
    m["kv41"] = kv

    P = np.zeros((41, 41), np.float32)
    for s in range(9):
        P[32 + s, s] = 1.0
        P[s, 32 + s] = 1.0
    m["perm"] = P

    rc41 = np.zeros((41,), np.float32)
    rc41[0:9] = rc9
    rc41[32:41] = rc9
    m["rc"] = rc41

    for name, v in (("g0", g0v), ("b0", b0v), ("g1", g1v), ("b1", b1v)):
        t = np.zeros((41, 64), np.float32)
        t[0:9] = np.asarray(v, np.float32)[None, :]
        t[32:41] = np.asarray(v, np.float32)[None, :]
        m[name] = t
    return m


LAST_RESULT = None


def kernel(features, ins_indices_batch, w0, g0, b0, w1, g1, b1):
    global LAST_RESULT
    _install_ntff_shim()
    from concourse.bass_utils import run_bass_kernel_spmd
    from concourse import bass2jax as _b2j
    import traceback as _tb

    _b2j.install_neuronx_cc_hook()
    import libneuronxla as _lnx

    if not getattr(_lnx, "_ant_dbg_wrapped", False):
        _orig = _lnx.neuronx_cc

        def _dbg(*a, **k):
            try:
                return _orig(*a, **k)
            except BaseException:
                _tb.print_exc()
                raise

        _lnx.neuronx_cc = _dbg
        _lnx._ant_dbg_wrapped = True

    x = np.asarray(features, np.float32)
    ids = np.asarray(ins_indices_batch).astype(np.int64)
    w0 = np.asarray(w0, np.float32)
    w1 = np.asarray(w1, np.float32)
    N = x.shape[0]
    H = x.shape[2]

    nc = build_nc(H)
    in_maps = [
        prep_core_inputs(x[i], ids[i], w0, g0, b0, w1, g1, b1, H) for i in range(N)
    ]
    trace = bool(int(os.environ.get("BASS_KERNEL_TRACE", "0")))
    res = run_bass_kernel_spmd(nc, in_maps, list(range(N)), trace=trace)
    LAST_RESULT = res
    outs = [
        res.results[i]["out"].reshape(C, H, W).astype(np.float32) for i in range(N)
    ]
    return np.stack(outs, 0)


# revision 3
# speedup vs baseline: 1.0068x; 1.0068x over previous
"""Trainium2 Bass kernel v2 for nn_DensePoseV1ConvXGNInsHead:
2x (conv3x3 64->64 -> per-instance BN -> ReLU) on [8,64,256,256],
one image per NeuronCore across 8 cores.

v2 redesign vs baseline:
- conv matmuls use full 128-partition block-diagonal weights (one MM per
  tap for both row-halves) instead of 64x64 quadrant pairs
- stats finalize swaps partition halves with a PE permutation matmul
  (no serial SBUF->SBUF DMAs on the critical path)
- all masks host-prepared and DMA'd contiguously
- L0-normalize and L1-conv interleaved in emission order so PE never drains
- f16 output
"""
import os
import sys
import types

sys.path.insert(0, "/opt/trn_rl_repo")

import numpy as np

import concourse.bass as bass
import concourse.tile as tile
from concourse import mybir
from concourse.vector_clock import ScopedClock

f16 = mybir.dt.float16
f32 = mybir.dt.float32
ALU = mybir.AluOpType
ACTF = mybir.ActivationFunctionType

C = 64          # channels
W = 256         # image width
PITCH = 272     # padded row pitch (16 left pad + 256 data; borrows next row's pad)
LP = 16         # left pad elements
EPS = 1e-5

# ---------------------------------------------------------------------------
# walrus workaround: split the Tile exit-drain's sem waits (installed walrus
# rejects instructions with >2 sync waits)
# ---------------------------------------------------------------------------
_patched = False


def _install_tile_patch():
    global _patched
    if _patched:
        return
    _patched = True

    def _drain_and_barrier(self, tick_clock, wait_clock):
        nc = self.nc
        drain_inst = nc.sync.drain()
        wait_clock.add_sem_waits(
            drain_inst.ins, ScopedClock({None: tick_clock.global_clock})
        )
        si = drain_inst.ins.sync_info
        waits = list(si.on_wait or [])
        if len(waits) > 1:
            si.on_wait = waits[:1]
            for i in range(1, len(waits)):
                nop = nc.sync.nop()
                nop.ins.sync_info = mybir.SyncInfo(
                    on_wait=waits[i : i + 1], on_update=[]
                )
        nc.all_engine_barrier()
        popped = nc._tile_sem_poison_stack.pop()
        assert popped is self._sem_poison
        nc.clear_and_free_semaphores(list(self.sems.allocated().values()))
        nc.all_engine_barrier()

    tile.TileContext._drain_and_barrier = _drain_and_barrier


# ---------------------------------------------------------------------------
# NTFF profiling shim (antenv.axon_hooks is absent in this image)
# ---------------------------------------------------------------------------
def _install_ntff_shim():
    if "antenv.axon_hooks" in sys.modules:
        return
    mod = types.ModuleType("antenv.axon_hooks")
    state = {"hook": None}
    mod.set_axon_ntff_profile_hook = lambda h: state.__setitem__("hook", h)
    mod.get_axon_ntff_profile_hook = lambda: state["hook"]
    sys.modules["antenv.axon_hooks"] = mod
    try:
        import antenv

        antenv.axon_hooks = mod
    except ImportError:
        pass
    try:
        from trn_agent_boot.trn_boot import _ntff_profile_via_ctypes

        h = _ntff_profile_via_ctypes("/opt/axon/libaxon_pjrt.so")
        mod.set_axon_ntff_profile_hook(h)
    except Exception:
        pass


def yoff(slot):
    return slot * PITCH + LP


def _ap(base_ap, offset_elems, dims):
    """Sub-AP of base_ap at +offset (elements) with given free dims."""
    return bass.AP(
        tensor=base_ap.tensor,
        offset=base_ap.offset + offset_elems,
        ap=[base_ap.ap[0]] + dims,
    )


def emit(nc, H=256):
    HH = H // 2          # 128 rows per half
    NCH = HH // 2        # conv chunks (2 rows) per layer = 64
    NPC = NCH * 4        # pixel chunks (128 px) per layer = 256
    NB = HH // 4         # x-load blocks (4 rows) = 32
    HW2 = HH * W

    xh = nc.declare_dram_parameter("xh", [C, (H + 1) * PITCH], f16, isOutput=False)
    w0d = nc.declare_dram_parameter("w0d", [128, 9, 128], f16, isOutput=False)
    w1d = nc.declare_dram_parameter("w1d", [128, 9, 128], f16, isOutput=False)
    id128 = nc.declare_dram_parameter("id128", [128, 128], f16, isOutput=False)
    idp_d = nc.declare_dram_parameter("idp", [128, 2 * NPC], f16, isOutput=False)
    krow_d = nc.declare_dram_parameter("krow", [128, 9], f16, isOutput=False)
    idsf_d = nc.declare_dram_parameter("idsf", [2, HW2], f16, isOutput=False)
    kv41_d = nc.declare_dram_parameter("kv41", [41], f32, isOutput=False)
    perm_d = nc.declare_dram_parameter("perm", [41, 41], f32, isOutput=False)
    rc_d = nc.declare_dram_parameter("rc", [41], f32, isOutput=False)
    g0d = nc.declare_dram_parameter("g0", [41, 64], f32, isOutput=False)
    b0d = nc.declare_dram_parameter("b0", [41, 64], f32, isOutput=False)
    g1d = nc.declare_dram_parameter("g1", [41, 64], f32, isOutput=False)
    b1d = nc.declare_dram_parameter("b1", [41, 64], f32, isOutput=False)
    outq = [
        nc.declare_dram_parameter(f"out{q}", [C, (H // 4) * W], f16, isOutput=True)
        for q in range(4)
    ]

    with tile.TileContext(nc) as tc:
        import contextlib

        with contextlib.ExitStack() as ctx:
            const = ctx.enter_context(tc.tile_pool(name="const", bufs=1))
            xbp = ctx.enter_context(tc.tile_pool(name="xbp", bufs=1))
            spp = ctx.enter_context(tc.tile_pool(name="spp", bufs=4))
            normp = ctx.enter_context(tc.tile_pool(name="normp", bufs=3))
            outp = ctx.enter_context(tc.tile_pool(name="outp", bufs=4))
            smallp = ctx.enter_context(tc.tile_pool(name="smallp", bufs=2))
            psc = ctx.enter_context(tc.tile_pool(name="psc", bufs=2, space="PSUM"))
            ptp = ctx.enter_context(tc.tile_pool(name="ptp", bufs=2, space="PSUM"))
            pss = ctx.enter_context(tc.tile_pool(name="pss", bufs=1, space="PSUM"))
            pse = ctx.enter_context(tc.tile_pool(name="pse", bufs=2, space="PSUM"))
            psw = ctx.enter_context(tc.tile_pool(name="psw", bufs=1, space="PSUM"))

            # ---- persistent y buffer (both layers, in-place)
            ysb = const.tile([128, (HH + 2) * PITCH + LP], f16)
            # zero only what stays zero: inter-row pads + top/bottom halo slots
            nc.vector.memset(_ap(ysb[:], 0, [[PITCH, HH + 3], [1, LP]]), 0.0)
            nc.vector.memset(_ap(ysb[:], yoff(0), [[1, W]]), 0.0)
            nc.vector.memset(_ap(ysb[:], yoff(HH + 1), [[1, W]]), 0.0)
            xb0 = xbp.tile([128, 6 * PITCH + LP], f16, tag="xb0")
            xb1 = xbp.tile([128, 6 * PITCH + LP], f16, tag="xb1")
            xbs = [xb0, xb1]

            # ---- constants from DRAM; only weights + identity loaded NOW.
            # Masks and finalize constants are DMA'd later (emission order =
            # queue order: they must not delay the first x loads).
            wts = []
            for wd in (w0d, w1d):
                wt = const.tile([128, 9, 128], f16, tag="wt", name=f"wt{len(wts)}")
                nc.sync.dma_start(out=wt[:], in_=wd[:])
                wts.append(wt)
            id128sb = const.tile([128, 128], f16)
            nc.sync.dma_start(out=id128sb[:], in_=id128[:])
            idp = const.tile([128, 2 * NPC], f16)
            nc.scalar.dma_start(out=idp[:], in_=idp_d[:])
            krow = const.tile([128, 9], f16)
            nc.scalar.dma_start(out=krow[:], in_=krow_d[:])
            # stats mask generated on-chip (saves 2.7MB of DMA):
            # cols 0:9 = (idA==k), 32:41 = (idB==k), 9:32 zero.
            # memset on DVE; compares on idle GPSIMD (keeps DVE free for
            # the first conv psum copies)
            mpm = const.tile([128, NPC, 41], f16)
            nc.vector.memset(_ap(mpm[:], 9, [[41, NPC], [1, 23]]), 0.0)
            nc.vector.tensor_tensor(
                _ap(mpm[:], 0, [[41, NPC], [1, 9]]),
                _ap(idp[:], 0, [[1, NPC], [0, 9]]),
                _ap(krow[:], 0, [[0, NPC], [1, 9]]),
                ALU.is_equal,
            )
            nc.vector.tensor_tensor(
                _ap(mpm[:], 32, [[41, NPC], [1, 9]]),
                _ap(idp[:], NPC, [[1, NPC], [0, 9]]),
                _ap(krow[:], 0, [[0, NPC], [1, 9]]),
                ALU.is_equal,
            )
            ms2sb = const.tile([41, HW2], f16)
            kv41 = const.tile([41, 1], f32)
            nc.scalar.dma_start(
                out=kv41[:], in_=kv41_d[:].rearrange("(a b) -> a b", b=1))
            MSL = HW2 // 16

            def gen_ms2_slice(k):
                """Fill ms2sb[:, k*MSL:(k+1)*MSL] from broadcast ids.

                Rows 0:9 <- (idA==s), rows 32:41 <- (idB==s); rows 9:32
                compare stale data against NaN -> always 0."""
                ids41 = spp.tile([41, MSL], f16, tag="ids41", name=f"ids41_{k}",
                                 bufs=2)
                nc.sync.dma_start(
                    out=ids41[0:9, :],
                    in_=bass.AP(tensor=idsf_d[:].tensor, offset=k * MSL,
                                ap=[[0, 9], [1, MSL]]),
                )
                nc.scalar.dma_start(
                    out=ids41[32:41, :],
                    in_=bass.AP(tensor=idsf_d[:].tensor, offset=HW2 + k * MSL,
                                ap=[[0, 9], [1, MSL]]),
                )
                nc.vector.tensor_scalar(
                    out=ms2sb[:, k * MSL : (k + 1) * MSL], in0=ids41[:],
                    scalar1=kv41[:], scalar2=None, op0=ALU.is_equal,
                )
            permsb = const.tile([41, 41], f32)
            rcsb = const.tile([41, 1], f32)
            gbt = []
            for li in (0, 1):
                gt = const.tile([41, 64], f32, tag="g18", name=f"g18_{li}")
                bt = const.tile([41, 64], f32, tag="b18", name=f"b18_{li}")
                gbt.append((gt, bt))
            epsap = const.tile([41, 1], f32)
            nc.vector.memset(epsap[:], EPS)

            def load_late_consts_2():
                # finalize constants (scalar DGE queue; needed ~180us in)
                nc.scalar.dma_start(out=permsb[:], in_=perm_d[:])
                nc.scalar.dma_start(
                    out=rcsb[:], in_=rc_d[:].rearrange("(a b) -> a b", b=1))
                for li, (gg, bb) in enumerate(((g0d, b0d), (g1d, b1d))):
                    nc.scalar.dma_start(out=gbt[li][0][:], in_=gg[:])
                    nc.scalar.dma_start(out=gbt[li][1][:], in_=bb[:])



            # ---- per-layer affine tables [18, 128] f16 (block structure):
            # rows 0:9 hold A-seg values in cols 0:64, rows 9:18 B-seg in 64:128.
            # Background rows 8 / 17: scale 1, offset 0 (set once, never touched).
            ones16 = const.tile([1, 64], f16)
            nc.vector.memset(ones16[:], 1.0)
            abts = []
            for L in (0, 1):
                s_t = const.tile([41, 128], f16, tag="abs", name=f"ab2s{L}")
                o_t = const.tile([41, 128], f16, tag="abo", name=f"ab2o{L}")
                nc.vector.memset(s_t[:], 0.0)
                nc.vector.memset(o_t[:], 0.0)
                # background rows (seg 8): scale 1, offset 0
                nc.scalar.dma_start(out=s_t[8:9, 0:64], in_=ones16[:])
                nc.scalar.dma_start(out=s_t[40:41, 64:128], in_=ones16[:])
                abts.append((s_t, o_t))

            # =============================================================
            # emission helpers
            # =============================================================
            def load_x_block(b):
                """Load x rows for 4-row block b into xb[b%2] (rows r0-1..r0+4).

                xh is PITCH-padded in DRAM, so each half loads as ONE fully
                contiguous run per channel (3.3KB packets, pads included)."""
                xb = xbs[b % 2]
                r0 = 4 * b
                XP = (H + 1) * PITCH

                def half_load(part_lo, part_hi, xb_off, row_lo, n_elems, coff):
                    eng = nc.sync if part_lo == 0 else nc.scalar
                    eng.dma_start(
                        out=_ap(xb[part_lo:part_hi, :], xb_off, [[1, n_elems]]),
                        in_=bass.AP(
                            tensor=xh[:].tensor,
                            offset=coff + row_lo * PITCH,
                            ap=[[XP, 64], [1, n_elems]],
                        ),
                    )

                if b == 0:
                    nc.vector.memset(xb[0:64, 0:PITCH], 0.0)
                    # A rows 0..4 -> slots 1..5 (incl. borrow tail)
                    half_load(0, 64, PITCH, 0, 5 * PITCH + LP, 0)
                    # B rows HH-1..HH+4 -> slots 0..5
                    half_load(64, 128, 0, HH - 1, 6 * PITCH + LP, 0)
                elif b == NB - 1:
                    # A rows r0-1..r0+4 (row r0+4 == HH exists)
                    half_load(0, 64, 0, r0 - 1, 6 * PITCH + LP, 0)
                    nc.vector.memset(
                        xb[64:128, 5 * PITCH : 6 * PITCH + LP], 0.0)
                    # B rows HH+r0-1..HH+r0+3 -> slots 0..4
                    half_load(64, 128, 0, HH + r0 - 1, 5 * PITCH + LP, 0)
                else:
                    half_load(0, 64, 0, r0 - 1, 6 * PITCH + LP, 0)
                    half_load(64, 128, 0, HH + r0 - 1, 6 * PITCH + LP, 0)

            def conv_chunk(L, c):
                """9-tap conv for 2-row chunk c (both halves) + copy to ysb."""
                wt = wts[L]
                slot0 = 1 if L == 0 else 0
                rr = 2 * c
                pts = psc.tile([128, 512], f32, tag="cps", name=f"cps_{L}_{c}")
                if L == 0:
                    src_t = xbs[(c // 2) % 2]
                    base_slot = 2 * (c % 2) + 1  # row rr at block slot
                else:
                    src_t = ysb
                    base_slot = rr + 1
                for t in range(9):
                    dy, dx = t // 3 - 1, t % 3 - 1
                    off = yoff(base_slot + dy) + dx
                    rhs = _ap(src_t[:], off, [[PITCH, 2], [1, W]])
                    nc.tensor.matmul(
                        pts[:], _ap(wt[:], t * 128, [[1, 128]]), rhs,
                        start=(t == 0), stop=(t == 8),
                    )
                dst = _ap(ysb[:], yoff(rr + slot0), [[PITCH, 2], [1, W]])
                nc.vector.tensor_copy(dst, pts[:])

            def stats_transpose(L, c):
                """PE-transpose the 4 128-px chunks of conv chunk c."""
                slot0 = 1 if L == 0 else 0
                pts2 = ptp.tile([128, 512], f16, tag="tp", name=f"tp_{L}_{c}")
                for j in range(4):
                    src = _ap(
                        ysb[:], yoff(2 * c + j // 2 + slot0) + (j % 2) * 128,
                        [[1, 128]],
                    )
                    nc.tensor.transpose(
                        pts2[:, j * 128 : (j + 1) * 128], src, id128sb[:]
                    )
                return pts2

            def stats_strip(L, c, pts2):
                """Copy transposed y to SBUF and square it."""
                sp = spp.tile([128, 1024], f16, tag="strip", name=f"sp_{L}_{c}")
                nc.scalar.copy(out=sp[:, 0:512], in_=pts2[:])
                nc.vector.tensor_tensor(
                    sp[:, 512:1024], sp[:, 0:512], sp[:, 0:512], ALU.mult
                )
                return sp

            def stats_mm(L, c, sp, stats):
                for j in range(4):
                    ci = 4 * c + j
                    nc.tensor.matmul(
                        stats[:],
                        _ap(mpm[:], ci * 41, [[1, 41]]),
                        _ap(sp[:], j * 128, [[512, 2], [1, 128]]),
                        start=(ci == 0), stop=(ci == NPC - 1),
                    )

            def finalize(L, stats):
                """stats psum [41,256] -> affine tables ab2s/ab2o for layer L.

                Rows 0:9 = A-half segment sums, rows 32:41 = B-half (32-aligned
                partition bases for engine-op legality); rows 9:32 stay zero."""
                gt, bt = gbt[L]
                s_t, o_t = abts[L]
                ssb = smallp.tile([41, 256], f32, tag="ssb", name=f"ssb{L}")
                nc.scalar.copy(out=ssb[:], in_=stats[:])
                # PE permutation: swp[s] <-> ssb[32+s] (A/B half swap)
                swp = psw.tile([41, 256], f32, tag="swp", name=f"swp{L}")
                nc.tensor.matmul(swp[:], permsb[:], ssb[:], start=True, stop=True)
                s1 = smallp.tile([41, 64], f32, tag="s1", name=f"s1_{L}")
                s2 = smallp.tile([41, 64], f32, tag="s2", name=f"s2_{L}")
                nc.vector.memset(s1[:], 0.0)
                nc.vector.memset(s2[:], 0.0)
                # own half data in own cols; other half arrives swapped
                nc.vector.tensor_tensor(
                    s1[0:9, :], ssb[0:9, 0:64], swp[0:9, 64:128], ALU.add)
                nc.vector.tensor_tensor(
                    s1[32:41, :], ssb[32:41, 64:128], swp[32:41, 0:64], ALU.add)
                nc.vector.tensor_tensor(
                    s2[0:9, :], ssb[0:9, 128:192], swp[0:9, 192:256], ALU.add)
                nc.vector.tensor_tensor(
                    s2[32:41, :], ssb[32:41, 192:256], swp[32:41, 128:192], ALU.add)
                mean = smallp.tile([41, 64], f32, tag="mean", name=f"mean{L}")
                nc.vector.tensor_scalar_mul(out=mean[:], in0=s1[:], scalar1=rcsb[:])
                e2 = smallp.tile([41, 64], f32, tag="e2", name=f"e2_{L}")
                nc.vector.tensor_scalar_mul(out=e2[:], in0=s2[:], scalar1=rcsb[:])
                var = smallp.tile([41, 64], f32, tag="var", name=f"var{L}")
                nc.vector.tensor_tensor(var[:], mean[:], mean[:], ALU.mult)
                nc.vector.tensor_tensor(var[:], e2[:], var[:], ALU.subtract)
                sd = smallp.tile([41, 64], f32, tag="sd", name=f"sd{L}")
                nc.scalar.activation(
                    out=sd[:], in_=var[:], func=ACTF.Sqrt, bias=epsap[:], scale=1.0
                )
                rstd = smallp.tile([41, 64], f32, tag="rstd", name=f"rstd{L}")
                nc.vector.reciprocal(out=rstd[:], in_=sd[:])
                a = smallp.tile([41, 64], f32, tag="a", name=f"a{L}")
                nc.vector.tensor_tensor(a[:], rstd[:], gt[:], ALU.mult)
                am = smallp.tile([41, 64], f32, tag="am", name=f"am{L}")
                nc.vector.tensor_tensor(am[:], a[:], mean[:], ALU.mult)
                bv = smallp.tile([41, 64], f32, tag="bv", name=f"bv{L}")
                nc.vector.tensor_tensor(bv[:], bt[:], am[:], ALU.subtract)
                # quadrant placement (aligned bases; skip bg rows 8/40)
                nc.vector.tensor_copy(s_t[0:8, 0:64], a[0:8, :])
                nc.scalar.copy(out=s_t[32:40, 64:128], in_=a[32:40, :])
                nc.vector.tensor_copy(o_t[0:8, 0:64], bv[0:8, :])
                nc.scalar.copy(out=o_t[32:40, 64:128], in_=bv[32:40, :])

            def norm_group(L, g):
                """Normalize+ReLU rows 2g,2g+1 (both halves) of layer L.

                L=1 (tail phase, no conv to overlap): sE copy goes to DVE and
                oE lives in the psc pool (idle in phase C) for deeper overlap."""
                s_t, o_t = abts[L]
                slot0 = 1 if L == 0 else 0
                win = ms2sb[:, g * 512 : (g + 1) * 512]
                sE = pse.tile([128, 512], f32, tag="exp", name=f"se{L}_{g}")
                if L == 0:
                    oE = pse.tile([128, 512], f32, tag="exp", name=f"oe{L}_{g}")
                else:
                    oE = psc.tile([128, 512], f32, tag="cps", name=f"oe{L}_{g}")
                nc.tensor.matmul(sE[:], s_t[:], win, start=True, stop=True)
                nc.tensor.matmul(oE[:], o_t[:], win, start=True, stop=True)
                sEc = normp.tile([128, 512], f16, tag="sEc", name=f"sec{L}_{g}")
                oEc = normp.tile([128, 512], f16, tag="oEc", name=f"oec{L}_{g}")
                if L == 0:
                    nc.scalar.copy(out=sEc[:], in_=sE[:])
                else:
                    nc.vector.tensor_copy(sEc[:], sE[:])
                nc.scalar.copy(out=oEc[:], in_=oE[:])
                yv = _ap(ysb[:], yoff(2 * g + slot0), [[PITCH, 2], [1, W]])
                t1 = normp.tile([128, 512], f16, tag="t1", name=f"t1_{L}_{g}")
                u = normp.tile([128, 512], f16, tag="u", name=f"u_{L}_{g}")
                nc.vector.tensor_tensor(t1[:], yv, sEc[:], ALU.mult)
                nc.vector.tensor_tensor(u[:], t1[:], oEc[:], ALU.add)
                if L == 0:
                    dst = yv
                else:
                    st = outp.tile([128, 512], f16, tag="st", name=f"st{g}")
                    dst = st[:]
                if L == 0 and g % 2 == 0:
                    nc.vector.tensor_scalar_max(out=dst, in0=u[:], scalar1=0.0)
                else:
                    # L=1: DVE already carries the sE copy; relu stays on ACT
                    nc.scalar.activation(out=dst, in_=u[:], func=ACTF.Relu)
                if L == 1:
                    # quarter q holds image rows [q*64, q*64+64); A-half rows
                    # 2g,2g+1 land in quarter g//32, B-half in 2 + g//32
                    qa = g // 32
                    go = g % 32
                    eng = nc.sync if g % 2 == 0 else nc.scalar
                    HQ = (H // 4) * W
                    eng.dma_start(
                        out=bass.AP(
                            tensor=outq[qa][:].tensor, offset=2 * go * W,
                            ap=[[HQ, 64], [W, 2], [1, W]],
                        ),
                        in_=st[0:64, :],
                    )
                    eng.dma_start(
                        out=bass.AP(
                            tensor=outq[2 + qa][:].tensor, offset=2 * go * W,
                            ap=[[HQ, 64], [W, 2], [1, W]],
                        ),
                        in_=st[64:128, :],
                    )

            # =============================================================
            # phase A: layer-0 conv + stats (software-pipelined)
            # =============================================================
            stats0 = pss.tile([41, 256], f32, tag="stats", name="stats0")
            load_x_block(0)
            load_x_block(1)
            pend = {}
            for c in range(NCH):
                conv_chunk(0, c)
                if c == 2:
                    load_late_consts_2()
                if 6 <= c < 38 and c % 2 == 0:
                    gen_ms2_slice((c - 6) // 2)
                if c % 2 == 1 and c // 2 + 2 < NB:
                    # prefetch after block c//2's last consumer (WAR via emission order)
                    load_x_block(c // 2 + 2)
                if c >= 1:
                    p2 = stats_transpose(0, c - 1)
                    pend[c - 1] = stats_strip(0, c - 1, p2)
                if c >= 4:
                    stats_mm(0, c - 4, pend.pop(c - 4), stats0)
            p2 = stats_transpose(0, NCH - 1)
            pend[NCH - 1] = stats_strip(0, NCH - 1, p2)
            for cc in range(NCH - 4, NCH):
                stats_mm(0, cc, pend.pop(cc), stats0)

            finalize(0, stats0)

            # =============================================================
            # phase B: layer-0 normalize interleaved with layer-1 conv+stats
            # norm group order: [63, 0, 1, ..., 62] so halos are ready early
            # =============================================================
            stats1 = pss.tile([41, 256], f32, tag="stats", name="stats1")
            G = [NCH - 1] + list(range(NCH - 1))
            LAG = 4
            for i in range(NCH + LAG):
                if i < NCH:
                    norm_group(0, G[i])
                if i == 0:
                    # B-half bottom: B slot 0 <- A row HH-1 (slot HH)
                    nc.sync.dma_start(
                        out=_ap(ysb[64:128, :], yoff(0), [[1, W]]),
                        in_=_ap(ysb[0:64, :], yoff(HH), [[1, W]]),
                    )
                if i == 1:
                    # A-half top halo: A slot HH+1 <- B row 0 (slot 1)
                    nc.sync.dma_start(
                        out=_ap(ysb[0:64, :], yoff(HH + 1), [[1, W]]),
                        in_=_ap(ysb[64:128, :], yoff(1), [[1, W]]),
                    )
                if i >= LAG:
                    c = i - LAG
                    conv_chunk(1, c)
                    if c >= 1:
                        p2 = stats_transpose(1, c - 1)
                        pend[c - 1] = stats_strip(1, c - 1, p2)
                    if c >= 4:
                        stats_mm(1, c - 4, pend.pop(c - 4), stats1)
            p2 = stats_transpose(1, NCH - 1)
            pend[NCH - 1] = stats_strip(1, NCH - 1, p2)
            for cc in range(NCH - 4, NCH):
                stats_mm(1, cc, pend.pop(cc), stats1)

            finalize(1, stats1)

            # =============================================================
            # phase C: layer-1 normalize -> out
            # =============================================================
            for g in range(NCH):
                norm_group(1, g)

    return nc


MAXW = 1


def _split_multi_waits(nc):
    """The installed walrus rejects instructions with >MAXW sync waits; hoist
    excess waits onto preceding same-engine nops."""
    nsplit = 0
    for fn in nc.m.functions:
        for blk in fn.blocks:
            insts = list(blk.instructions)
            out = []
            for inst in insts:
                si = inst.sync_info
                waits = list(si.on_wait) if (si and si.on_wait) else []
                if len(waits) > MAXW:
                    for i in range(0, len(waits) - MAXW, MAXW):
                        nop = mybir.InstNoOp(
                            name=f"WSPLIT-{nsplit}", ins=[], outs=[]
                        )
                        nsplit += 1
                        nop.engine = inst.engine
                        nop.sync_info = mybir.SyncInfo(
                            on_wait=waits[i : i + MAXW], on_update=[]
                        )
                        out.append(nop)
                    si.on_wait = waits[len(waits) - MAXW :]
                out.append(inst)
            if len(out) != len(insts):
                while len(blk.instructions):
                    blk.instructions.pop()
                for inst in out:
                    blk.instructions.append(inst)
    return nsplit


def build_nc(H=256, split_waits=True):
    _install_tile_patch()
    nc = bass.Bass()
    emit(nc, H)
    if split_waits:
        n = _split_multi_waits(nc)
        if n:
            print(f"kernel: split {n} multi-wait instructions")
    return nc


# ---------------------------------------------------------------------------
# host-side input prep
# ---------------------------------------------------------------------------
def prep_core_inputs(x_img, ids_img, w0, g0v, b0v, w1, g1v, b1v, H=256):
    HH = H // 2
    NPC = HH * 2
    HW2 = HH * W
    seg = np.where(ids_img < 0, 8, ids_img).astype(np.int64)
    segA, segB = seg[:HH], seg[HH:]

    m = {}
    xpad = np.zeros((C, (H + 1) * PITCH), np.float16)
    xr = xpad.reshape(C, H + 1, PITCH)
    xr[:, :H, LP : LP + W] = x_img.astype(np.float16)
    m["xh"] = np.ascontiguousarray(xpad)
    cnt = np.bincount(seg.reshape(-1), minlength=9)[:9]
    rc9 = (1.0 / np.maximum(cnt, 1)).astype(np.float32)
    
    for name, wmat in (("w0d", w0), ("w1d", w1)):
        wd = np.zeros((9, 128, 128), np.float16)
        for t in range(9):
            dy, dx = t // 3, t % 3
            lhsT = wmat[:, :, dy, dx].T.astype(np.float16)  # [cin, cout]
            wd[t, 0:64, 0:64] = lhsT
            wd[t, 64:128, 64:128] = lhsT
        m[name] = np.ascontiguousarray(wd.transpose(1, 0, 2))  # [ci, t, co]

    m["id128"] = np.eye(128, dtype=np.float16)

    # pixel-major ids [128, 2, NPC]: [:,0,ci]=A-seg, [:,1,ci]=B-seg of the
    # pixel (r, cs*128+p), ci = 2*r + cs; stats mask built on-chip from this
    pA = segA.reshape(HH, 2, 128).transpose(2, 0, 1).reshape(128, NPC)
    pB = segB.reshape(HH, 2, 128).transpose(2, 0, 1).reshape(128, NPC)
    m["idp"] = np.ascontiguousarray(
        np.stack([pA, pB], 1).reshape(128, 2 * NPC).astype(np.float16))
    m["krow"] = np.ascontiguousarray(
        np.broadcast_to(np.arange(9, dtype=np.float16), (128, 9))).copy()

    # raw ids for on-chip expansion-mask build
    m["idsf"] = np.ascontiguousarray(
        np.stack([segA.reshape(-1), segB.reshape(-1)]).astype(np.float16))
    kv = np.full(41, np.nan, np.float32)
    kv[0:9] = np.arange(9)
    kv[32:41] = np.arange(9)
    m["kv41"] = kv

    P = np.zeros((41, 41), np.float32)
    for s in range(9):
        P[32 + s, s] = 1.0
        P[s, 32 + s] = 1.0
    m["perm"] = P

    rc41 = np.zeros((41,), np.float32)
    rc41[0:9] = rc9
    rc41[32:41] = rc9
    m["rc"] = rc41

    for name, v in (("g0", g0v), ("b0", b0v), ("g1", g1v), ("b1", b1v)):
        t = np.zeros((41, 64), np.float32)
        t[0:9] = np.asarray(v, np.float32)[None, :]
        t[32:41] = np.asarray(v, np.float32)[None, :]
        m[name] = t
    return m


LAST_RESULT = None


def kernel(features, ins_indices_batch, w0, g0, b0, w1, g1, b1):
    global LAST_RESULT
    _install_ntff_shim()
    from concourse.bass_utils import run_bass_kernel_spmd
    from concourse import bass2jax as _b2j
    import traceback as _tb

    _b2j.install_neuronx_cc_hook()
    import libneuronxla as _lnx

    if not getattr(_lnx, "_ant_dbg_wrapped", False):
        _orig = _lnx.neuronx_cc

        def _dbg(*a, **k):
            try:
                return _orig(*a, **k)
            except BaseException:
                _tb.print_exc()
                raise

        _lnx.neuronx_cc = _dbg
        _lnx._ant_dbg_wrapped = True

    x = np.asarray(features, np.float32)
    ids = np.asarray(ins_indices_batch).astype(np.int64)
    w0 = np.asarray(w0, np.float32)
    w1 = np.asarray(w1, np.float32)
    N = x.shape[0]
    H = x.shape[2]

    nc = build_nc(H)
    in_maps = [
        prep_core_inputs(x[i], ids[i], w0, g0, b0, w1, g1, b1, H) for i in range(N)
    ]
    trace = bool(int(os.environ.get("BASS_KERNEL_TRACE", "0")))
    res = run_bass_kernel_spmd(nc, in_maps, list(range(N)), trace=trace)
    LAST_RESULT = res
    outs = []
    for i in range(N):
        qs = [res.results[i][f"out{q}"].reshape(C, H // 4, W) for q in range(4)]
        outs.append(np.concatenate(qs, axis=1).astype(np.float32))
    return np.stack(outs, 0)


# revision 4
# speedup vs baseline: 1.0170x; 1.0101x over previous
"""Trainium2 Bass kernel v2 for nn_DensePoseV1ConvXGNInsHead:
2x (conv3x3 64->64 -> per-instance BN -> ReLU) on [8,64,256,256],
one image per NeuronCore across 8 cores.

v2 redesign vs baseline:
- conv matmuls use full 128-partition block-diagonal weights (one MM per
  tap for both row-halves) instead of 64x64 quadrant pairs
- stats finalize swaps partition halves with a PE permutation matmul
  (no serial SBUF->SBUF DMAs on the critical path)
- all masks host-prepared and DMA'd contiguously
- L0-normalize and L1-conv interleaved in emission order so PE never drains
- f16 output
"""
import os
import sys
import types

sys.path.insert(0, "/opt/trn_rl_repo")

import numpy as np

import concourse.bass as bass
import concourse.tile as tile
from concourse import mybir
from concourse.vector_clock import ScopedClock

f16 = mybir.dt.float16
f32 = mybir.dt.float32
ALU = mybir.AluOpType
ACTF = mybir.ActivationFunctionType

C = 64          # channels
W = 256         # image width
PITCH = 272     # padded row pitch (16 left pad + 256 data; borrows next row's pad)
LP = 16         # left pad elements
EPS = 1e-5

# ---------------------------------------------------------------------------
# walrus workaround: split the Tile exit-drain's sem waits (installed walrus
# rejects instructions with >2 sync waits)
# ---------------------------------------------------------------------------
_patched = False


def _install_tile_patch():
    global _patched
    if _patched:
        return
    _patched = True

    def _drain_and_barrier(self, tick_clock, wait_clock):
        nc = self.nc
        drain_inst = nc.sync.drain()
        wait_clock.add_sem_waits(
            drain_inst.ins, ScopedClock({None: tick_clock.global_clock})
        )
        si = drain_inst.ins.sync_info
        waits = list(si.on_wait or [])
        if len(waits) > 1:
            si.on_wait = waits[:1]
            for i in range(1, len(waits)):
                nop = nc.sync.nop()
                nop.ins.sync_info = mybir.SyncInfo(
                    on_wait=waits[i : i + 1], on_update=[]
                )
        nc.all_engine_barrier()
        popped = nc._tile_sem_poison_stack.pop()
        assert popped is self._sem_poison
        nc.clear_and_free_semaphores(list(self.sems.allocated().values()))
        nc.all_engine_barrier()

    tile.TileContext._drain_and_barrier = _drain_and_barrier


# ---------------------------------------------------------------------------
# NTFF profiling shim (antenv.axon_hooks is absent in this image)
# ---------------------------------------------------------------------------
def _install_ntff_shim():
    if "antenv.axon_hooks" in sys.modules:
        return
    mod = types.ModuleType("antenv.axon_hooks")
    state = {"hook": None}
    mod.set_axon_ntff_profile_hook = lambda h: state.__setitem__("hook", h)
    mod.get_axon_ntff_profile_hook = lambda: state["hook"]
    sys.modules["antenv.axon_hooks"] = mod
    try:
        import antenv

        antenv.axon_hooks = mod
    except ImportError:
        pass
    try:
        from trn_agent_boot.trn_boot import _ntff_profile_via_ctypes

        h = _ntff_profile_via_ctypes("/opt/axon/libaxon_pjrt.so")
        mod.set_axon_ntff_profile_hook(h)
    except Exception:
        pass


def yoff(slot):
    return slot * PITCH + LP


def _ap(base_ap, offset_elems, dims):
    """Sub-AP of base_ap at +offset (elements) with given free dims."""
    return bass.AP(
        tensor=base_ap.tensor,
        offset=base_ap.offset + offset_elems,
        ap=[base_ap.ap[0]] + dims,
    )


def emit(nc, H=256):
    HH = H // 2          # 128 rows per half
    NCH = HH // 2        # conv chunks (2 rows) per layer = 64
    NPC = NCH * 4        # pixel chunks (128 px) per layer = 256
    NB = HH // 4         # x-load blocks (4 rows) = 32
    HW2 = HH * W

    xh = nc.declare_dram_parameter("xh", [C, (H + 1) * PITCH], f16, isOutput=False)
    w0d = nc.declare_dram_parameter("w0d", [128, 9, 128], f16, isOutput=False)
    w1d = nc.declare_dram_parameter("w1d", [128, 9, 128], f16, isOutput=False)
    id128 = nc.declare_dram_parameter("id128", [128, 128], f16, isOutput=False)
    idp_d = nc.declare_dram_parameter("idp", [128, 2 * NPC], f16, isOutput=False)
    krow_d = nc.declare_dram_parameter("krow", [128, 9], f16, isOutput=False)
    idsf_d = nc.declare_dram_parameter("idsf", [2, HW2], f16, isOutput=False)
    kv41_d = nc.declare_dram_parameter("kv41", [41], f32, isOutput=False)
    perm_d = nc.declare_dram_parameter("perm", [41, 41], f32, isOutput=False)
    rc_d = nc.declare_dram_parameter("rc", [41], f32, isOutput=False)
    g0d = nc.declare_dram_parameter("g0", [41, 64], f32, isOutput=False)
    b0d = nc.declare_dram_parameter("b0", [41, 64], f32, isOutput=False)
    g1d = nc.declare_dram_parameter("g1", [41, 64], f32, isOutput=False)
    b1d = nc.declare_dram_parameter("b1", [41, 64], f32, isOutput=False)
    outq = [
        nc.declare_dram_parameter(f"out{q}", [C, (H // 4) * W], f16, isOutput=True)
        for q in range(4)
    ]

    with tile.TileContext(nc) as tc:
        import contextlib

        with contextlib.ExitStack() as ctx:
            const = ctx.enter_context(tc.tile_pool(name="const", bufs=1))
            xbp = ctx.enter_context(tc.tile_pool(name="xbp", bufs=1))
            spp = ctx.enter_context(tc.tile_pool(name="spp", bufs=5))
            normp = ctx.enter_context(tc.tile_pool(name="normp", bufs=3))
            outp = ctx.enter_context(tc.tile_pool(name="outp", bufs=4))
            smallp = ctx.enter_context(tc.tile_pool(name="smallp", bufs=2))
            psc = ctx.enter_context(tc.tile_pool(name="psc", bufs=2, space="PSUM"))
            ptp = ctx.enter_context(tc.tile_pool(name="ptp", bufs=2, space="PSUM"))
            pss = ctx.enter_context(tc.tile_pool(name="pss", bufs=1, space="PSUM"))
            pse = ctx.enter_context(tc.tile_pool(name="pse", bufs=2, space="PSUM"))
            psw = ctx.enter_context(tc.tile_pool(name="psw", bufs=1, space="PSUM"))

            # ---- persistent y buffer (both layers, in-place)
            ysb = const.tile([128, (HH + 2) * PITCH + LP], f16)
            # zero only what stays zero: inter-row pads + top/bottom halo slots
            nc.vector.memset(_ap(ysb[:], 0, [[PITCH, HH + 3], [1, LP]]), 0.0)
            nc.vector.memset(_ap(ysb[:], yoff(0), [[1, W]]), 0.0)
            nc.vector.memset(_ap(ysb[:], yoff(HH + 1), [[1, W]]), 0.0)
            xb0 = xbp.tile([128, 6 * PITCH + LP], f16, tag="xb0")
            xb1 = xbp.tile([128, 6 * PITCH + LP], f16, tag="xb1")
            xbs = [xb0, xb1]

            # ---- constants from DRAM; only weights + identity loaded NOW.
            # Masks and finalize constants are DMA'd later (emission order =
            # queue order: they must not delay the first x loads).
            wts = []
            for wd in (w0d, w1d):
                wt = const.tile([128, 9, 128], f16, tag="wt", name=f"wt{len(wts)}")
                nc.sync.dma_start(out=wt[:], in_=wd[:])
                wts.append(wt)
            id128sb = const.tile([128, 128], f16)
            nc.sync.dma_start(out=id128sb[:], in_=id128[:])
            idp = const.tile([128, 2 * NPC], f16)
            nc.scalar.dma_start(out=idp[:], in_=idp_d[:])
            krow = const.tile([128, 9], f16)
            nc.scalar.dma_start(out=krow[:], in_=krow_d[:])
            # stats mask generated on-chip (saves 2.7MB of DMA):
            # cols 0:9 = (idA==k), 32:41 = (idB==k), 9:32 zero.
            # memset on DVE; compares on idle GPSIMD (keeps DVE free for
            # the first conv psum copies)
            mpm = const.tile([128, NPC, 41], f16)
            nc.vector.memset(_ap(mpm[:], 9, [[41, NPC], [1, 23]]), 0.0)
            nc.vector.tensor_tensor(
                _ap(mpm[:], 0, [[41, NPC], [1, 9]]),
                _ap(idp[:], 0, [[1, NPC], [0, 9]]),
                _ap(krow[:], 0, [[0, NPC], [1, 9]]),
                ALU.is_equal,
            )
            nc.vector.tensor_tensor(
                _ap(mpm[:], 32, [[41, NPC], [1, 9]]),
                _ap(idp[:], NPC, [[1, NPC], [0, 9]]),
                _ap(krow[:], 0, [[0, NPC], [1, 9]]),
                ALU.is_equal,
            )
            ms2sb = const.tile([41, HW2], f16)
            kv41 = const.tile([41, 1], f32)
            nc.scalar.dma_start(
                out=kv41[:], in_=kv41_d[:].rearrange("(a b) -> a b", b=1))
            MSL = HW2 // 16

            def gen_ms2_slice(k):
                """Fill ms2sb[:, k*MSL:(k+1)*MSL] from broadcast ids.

                Rows 0:9 <- (idA==s), rows 32:41 <- (idB==s); rows 9:32
                compare stale data against NaN -> always 0."""
                ids41 = spp.tile([41, MSL], f16, tag="ids41", name=f"ids41_{k}",
                                 bufs=2)
                nc.sync.dma_start(
                    out=ids41[0:9, :],
                    in_=bass.AP(tensor=idsf_d[:].tensor, offset=k * MSL,
                                ap=[[0, 9], [1, MSL]]),
                )
                nc.scalar.dma_start(
                    out=ids41[32:41, :],
                    in_=bass.AP(tensor=idsf_d[:].tensor, offset=HW2 + k * MSL,
                                ap=[[0, 9], [1, MSL]]),
                )
                nc.vector.tensor_scalar(
                    out=ms2sb[:, k * MSL : (k + 1) * MSL], in0=ids41[:],
                    scalar1=kv41[:], scalar2=None, op0=ALU.is_equal,
                )
            permsb = const.tile([41, 41], f32)
            rcsb = const.tile([41, 1], f32)
            gbt = []
            for li in (0, 1):
                gt = const.tile([41, 64], f32, tag="g18", name=f"g18_{li}")
                bt = const.tile([41, 64], f32, tag="b18", name=f"b18_{li}")
                gbt.append((gt, bt))
            epsap = const.tile([41, 1], f32)
            nc.vector.memset(epsap[:], EPS)

            def load_late_consts_2():
                # finalize constants (scalar DGE queue; needed ~180us in)
                nc.scalar.dma_start(out=permsb[:], in_=perm_d[:])
                nc.scalar.dma_start(
                    out=rcsb[:], in_=rc_d[:].rearrange("(a b) -> a b", b=1))
                for li, (gg, bb) in enumerate(((g0d, b0d), (g1d, b1d))):
                    nc.scalar.dma_start(out=gbt[li][0][:], in_=gg[:])
                    nc.scalar.dma_start(out=gbt[li][1][:], in_=bb[:])



            # ---- per-layer affine tables [18, 128] f16 (block structure):
            # rows 0:9 hold A-seg values in cols 0:64, rows 9:18 B-seg in 64:128.
            # Background rows 8 / 17: scale 1, offset 0 (set once, never touched).
            ones16 = const.tile([1, 64], f16)
            nc.vector.memset(ones16[:], 1.0)
            abts = []
            for L in (0, 1):
                s_t = const.tile([41, 128], f16, tag="abs", name=f"ab2s{L}")
                o_t = const.tile([41, 128], f16, tag="abo", name=f"ab2o{L}")
                nc.vector.memset(s_t[:], 0.0)
                nc.vector.memset(o_t[:], 0.0)
                # background rows (seg 8): scale 1, offset 0
                nc.scalar.dma_start(out=s_t[8:9, 0:64], in_=ones16[:])
                nc.scalar.dma_start(out=s_t[40:41, 64:128], in_=ones16[:])
                abts.append((s_t, o_t))

            # =============================================================
            # emission helpers
            # =============================================================
            def load_x_block(b):
                """Load x rows for 4-row block b into xb[b%2] (rows r0-1..r0+4).

                xh is PITCH-padded in DRAM, so each half loads as ONE fully
                contiguous run per channel (3.3KB packets, pads included)."""
                xb = xbs[b % 2]
                r0 = 4 * b
                XP = (H + 1) * PITCH

                def half_load(part_lo, part_hi, xb_off, row_lo, n_elems, coff):
                    eng = nc.sync if part_lo == 0 else nc.scalar
                    eng.dma_start(
                        out=_ap(xb[part_lo:part_hi, :], xb_off, [[1, n_elems]]),
                        in_=bass.AP(
                            tensor=xh[:].tensor,
                            offset=coff + row_lo * PITCH,
                            ap=[[XP, 64], [1, n_elems]],
                        ),
                    )

                if b == 0:
                    nc.vector.memset(xb[0:64, 0:PITCH], 0.0)
                    # A rows 0..4 -> slots 1..5 (incl. borrow tail)
                    half_load(0, 64, PITCH, 0, 5 * PITCH + LP, 0)
                    # B rows HH-1..HH+4 -> slots 0..5
                    half_load(64, 128, 0, HH - 1, 6 * PITCH + LP, 0)
                elif b == NB - 1:
                    # A rows r0-1..r0+4 (row r0+4 == HH exists)
                    half_load(0, 64, 0, r0 - 1, 6 * PITCH + LP, 0)
                    nc.vector.memset(
                        xb[64:128, 5 * PITCH : 6 * PITCH + LP], 0.0)
                    # B rows HH+r0-1..HH+r0+3 -> slots 0..4
                    half_load(64, 128, 0, HH + r0 - 1, 5 * PITCH + LP, 0)
                else:
                    half_load(0, 64, 0, r0 - 1, 6 * PITCH + LP, 0)
                    half_load(64, 128, 0, HH + r0 - 1, 6 * PITCH + LP, 0)

            def conv_chunk(L, c):
                """9-tap conv for 2-row chunk c (both halves) + copy to ysb."""
                wt = wts[L]
                slot0 = 1 if L == 0 else 0
                rr = 2 * c
                pts = psc.tile([128, 512], f32, tag="cps", name=f"cps_{L}_{c}")
                if L == 0:
                    src_t = xbs[(c // 2) % 2]
                    base_slot = 2 * (c % 2) + 1  # row rr at block slot
                else:
                    src_t = ysb
                    base_slot = rr + 1
                for t in range(9):
                    dy, dx = t // 3 - 1, t % 3 - 1
                    off = yoff(base_slot + dy) + dx
                    rhs = _ap(src_t[:], off, [[PITCH, 2], [1, W]])
                    nc.tensor.matmul(
                        pts[:], _ap(wt[:], t * 128, [[1, 128]]), rhs,
                        start=(t == 0), stop=(t == 8),
                    )
                dst = _ap(ysb[:], yoff(rr + slot0), [[PITCH, 2], [1, W]])
                nc.vector.tensor_copy(dst, pts[:])

            def stats_transpose(L, c):
                """PE-transpose the 4 128-px chunks of conv chunk c."""
                slot0 = 1 if L == 0 else 0
                pts2 = ptp.tile([128, 512], f16, tag="tp", name=f"tp_{L}_{c}")
                for j in range(4):
                    src = _ap(
                        ysb[:], yoff(2 * c + j // 2 + slot0) + (j % 2) * 128,
                        [[1, 128]],
                    )
                    nc.tensor.transpose(
                        pts2[:, j * 128 : (j + 1) * 128], src, id128sb[:]
                    )
                return pts2

            def stats_strip(L, c, pts2):
                """Copy transposed y to SBUF and square it."""
                sp = spp.tile([128, 1024], f16, tag="strip", name=f"sp_{L}_{c}")
                nc.scalar.copy(out=sp[:, 0:512], in_=pts2[:])
                nc.vector.tensor_tensor(
                    sp[:, 512:1024], sp[:, 0:512], sp[:, 0:512], ALU.mult
                )
                return sp

            def stats_mm(L, c, sp, stats):
                for j in range(4):
                    ci = 4 * c + j
                    nc.tensor.matmul(
                        stats[:],
                        _ap(mpm[:], ci * 41, [[1, 41]]),
                        _ap(sp[:], j * 128, [[512, 2], [1, 128]]),
                        start=(ci == 0), stop=(ci == NPC - 1),
                    )

            def finalize(L, stats):
                """stats psum [41,256] -> affine tables ab2s/ab2o for layer L.

                Rows 0:9 = A-half segment sums, rows 32:41 = B-half (32-aligned
                partition bases for engine-op legality); rows 9:32 stay zero."""
                gt, bt = gbt[L]
                s_t, o_t = abts[L]
                ssb = smallp.tile([41, 256], f32, tag="ssb", name=f"ssb{L}")
                nc.scalar.copy(out=ssb[:], in_=stats[:])
                # PE permutation: swp[s] <-> ssb[32+s] (A/B half swap)
                swp = psw.tile([41, 256], f32, tag="swp", name=f"swp{L}")
                nc.tensor.matmul(swp[:], permsb[:], ssb[:], start=True, stop=True)
                s1 = smallp.tile([41, 64], f32, tag="s1", name=f"s1_{L}")
                s2 = smallp.tile([41, 64], f32, tag="s2", name=f"s2_{L}")
                nc.vector.memset(s1[:], 0.0)
                nc.vector.memset(s2[:], 0.0)
                # own half data in own cols; other half arrives swapped
                nc.vector.tensor_tensor(
                    s1[0:9, :], ssb[0:9, 0:64], swp[0:9, 64:128], ALU.add)
                nc.vector.tensor_tensor(
                    s1[32:41, :], ssb[32:41, 64:128], swp[32:41, 0:64], ALU.add)
                nc.vector.tensor_tensor(
                    s2[0:9, :], ssb[0:9, 128:192], swp[0:9, 192:256], ALU.add)
                nc.vector.tensor_tensor(
                    s2[32:41, :], ssb[32:41, 192:256], swp[32:41, 128:192], ALU.add)
                mean = smallp.tile([41, 64], f32, tag="mean", name=f"mean{L}")
                nc.vector.tensor_scalar_mul(out=mean[:], in0=s1[:], scalar1=rcsb[:])
                e2 = smallp.tile([41, 64], f32, tag="e2", name=f"e2_{L}")
                nc.vector.tensor_scalar_mul(out=e2[:], in0=s2[:], scalar1=rcsb[:])
                var = smallp.tile([41, 64], f32, tag="var", name=f"var{L}")
                nc.vector.tensor_tensor(var[:], mean[:], mean[:], ALU.mult)
                nc.vector.tensor_tensor(var[:], e2[:], var[:], ALU.subtract)
                sd = smallp.tile([41, 64], f32, tag="sd", name=f"sd{L}")
                nc.scalar.activation(
                    out=sd[:], in_=var[:], func=ACTF.Sqrt, bias=epsap[:], scale=1.0
                )
                rstd = smallp.tile([41, 64], f32, tag="rstd", name=f"rstd{L}")
                nc.vector.reciprocal(out=rstd[:], in_=sd[:])
                a = smallp.tile([41, 64], f32, tag="a", name=f"a{L}")
                nc.vector.tensor_tensor(a[:], rstd[:], gt[:], ALU.mult)
                am = smallp.tile([41, 64], f32, tag="am", name=f"am{L}")
                nc.vector.tensor_tensor(am[:], a[:], mean[:], ALU.mult)
                bv = smallp.tile([41, 64], f32, tag="bv", name=f"bv{L}")
                nc.vector.tensor_tensor(bv[:], bt[:], am[:], ALU.subtract)
                # quadrant placement (aligned bases; skip bg rows 8/40)
                nc.vector.tensor_copy(s_t[0:8, 0:64], a[0:8, :])
                nc.scalar.copy(out=s_t[32:40, 64:128], in_=a[32:40, :])
                nc.vector.tensor_copy(o_t[0:8, 0:64], bv[0:8, :])
                nc.scalar.copy(out=o_t[32:40, 64:128], in_=bv[32:40, :])

            def norm_group(L, g):
                """Normalize+ReLU rows 2g,2g+1 (both halves) of layer L.

                L=1 (tail phase, no conv to overlap): sE copy goes to DVE and
                oE lives in the psc pool (idle in phase C) for deeper overlap."""
                s_t, o_t = abts[L]
                slot0 = 1 if L == 0 else 0
                win = ms2sb[:, g * 512 : (g + 1) * 512]
                sE = pse.tile([128, 512], f32, tag="exp", name=f"se{L}_{g}")
                if L == 0:
                    oE = pse.tile([128, 512], f32, tag="exp", name=f"oe{L}_{g}")
                else:
                    oE = psc.tile([128, 512], f32, tag="cps", name=f"oe{L}_{g}")
                nc.tensor.matmul(sE[:], s_t[:], win, start=True, stop=True)
                nc.tensor.matmul(oE[:], o_t[:], win, start=True, stop=True)
                sEc = normp.tile([128, 512], f16, tag="sEc", name=f"sec{L}_{g}")
                oEc = normp.tile([128, 512], f16, tag="oEc", name=f"oec{L}_{g}")
                if L == 0:
                    nc.scalar.copy(out=sEc[:], in_=sE[:])
                else:
                    nc.vector.tensor_copy(sEc[:], sE[:])
                nc.scalar.copy(out=oEc[:], in_=oE[:])
                yv = _ap(ysb[:], yoff(2 * g + slot0), [[PITCH, 2], [1, W]])
                t1 = normp.tile([128, 512], f16, tag="t1", name=f"t1_{L}_{g}")
                u = normp.tile([128, 512], f16, tag="u", name=f"u_{L}_{g}")
                nc.vector.tensor_tensor(t1[:], yv, sEc[:], ALU.mult)
                nc.vector.tensor_tensor(u[:], t1[:], oEc[:], ALU.add)
                if L == 0:
                    dst = yv
                else:
                    st = outp.tile([128, 512], f16, tag="st", name=f"st{g}")
                    dst = st[:]
                if L == 0 and g % 2 == 0:
                    nc.vector.tensor_scalar_max(out=dst, in0=u[:], scalar1=0.0)
                else:
                    # L=1: DVE already carries the sE copy; relu stays on ACT
                    nc.scalar.activation(out=dst, in_=u[:], func=ACTF.Relu)
                if L == 1:
                    # quarter q holds image rows [q*64, q*64+64); A-half rows
                    # 2g,2g+1 land in quarter g//32, B-half in 2 + g//32
                    qa = g // 32
                    go = g % 32
                    eng = nc.sync if g % 2 == 0 else nc.scalar
                    HQ = (H // 4) * W
                    eng.dma_start(
                        out=bass.AP(
                            tensor=outq[qa][:].tensor, offset=2 * go * W,
                            ap=[[HQ, 64], [W, 2], [1, W]],
                        ),
                        in_=st[0:64, :],
                    )
                    eng.dma_start(
                        out=bass.AP(
                            tensor=outq[2 + qa][:].tensor, offset=2 * go * W,
                            ap=[[HQ, 64], [W, 2], [1, W]],
                        ),
                        in_=st[64:128, :],
                    )

            # =============================================================
            # phase A: layer-0 conv + stats (software-pipelined)
            # =============================================================
            stats0 = pss.tile([41, 256], f32, tag="stats", name="stats0")
            load_x_block(0)
            load_x_block(1)
            pend = {}
            for c in range(NCH):
                conv_chunk(0, c)
                if c == 2:
                    load_late_consts_2()
                if 6 <= c < 38 and c % 2 == 0:
                    gen_ms2_slice((c - 6) // 2)
                if c % 2 == 1 and c // 2 + 2 < NB:
                    # prefetch after block c//2's last consumer (WAR via emission order)
                    load_x_block(c // 2 + 2)
                if c >= 1:
                    p2 = stats_transpose(0, c - 1)
                    pend[c - 1] = stats_strip(0, c - 1, p2)
                if c >= 5:
                    stats_mm(0, c - 5, pend.pop(c - 5), stats0)
            p2 = stats_transpose(0, NCH - 1)
            pend[NCH - 1] = stats_strip(0, NCH - 1, p2)
            for cc in range(NCH - 5, NCH):
                stats_mm(0, cc, pend.pop(cc), stats0)

            finalize(0, stats0)

            # =============================================================
            # phase B: layer-0 normalize interleaved with layer-1 conv+stats
            # norm group order: [63, 0, 1, ..., 62] so halos are ready early
            # =============================================================
            stats1 = pss.tile([41, 256], f32, tag="stats", name="stats1")
            G = [NCH - 1] + list(range(NCH - 1))
            LAG = 4
            for i in range(NCH + LAG):
                if i < NCH:
                    norm_group(0, G[i])
                if i == 0:
                    # B-half bottom: B slot 0 <- A row HH-1 (slot HH)
                    nc.sync.dma_start(
                        out=_ap(ysb[64:128, :], yoff(0), [[1, W]]),
                        in_=_ap(ysb[0:64, :], yoff(HH), [[1, W]]),
                    )
                if i == 1:
                    # A-half top halo: A slot HH+1 <- B row 0 (slot 1)
                    nc.sync.dma_start(
                        out=_ap(ysb[0:64, :], yoff(HH + 1), [[1, W]]),
                        in_=_ap(ysb[64:128, :], yoff(1), [[1, W]]),
                    )
                if i >= LAG:
                    c = i - LAG
                    conv_chunk(1, c)
                    if c >= 1:
                        p2 = stats_transpose(1, c - 1)
                        pend[c - 1] = stats_strip(1, c - 1, p2)
                    if c >= 5:
                        stats_mm(1, c - 5, pend.pop(c - 5), stats1)
            p2 = stats_transpose(1, NCH - 1)
            pend[NCH - 1] = stats_strip(1, NCH - 1, p2)
            for cc in range(NCH - 5, NCH):
                stats_mm(1, cc, pend.pop(cc), stats1)

            finalize(1, stats1)

            # =============================================================
            # phase C: layer-1 normalize -> out
            # =============================================================
            for g in range(NCH):
                norm_group(1, g)

    return nc


MAXW = 1


def _split_multi_waits(nc):
    """The installed walrus rejects instructions with >MAXW sync waits; hoist
    excess waits onto preceding same-engine nops."""
    nsplit = 0
    for fn in nc.m.functions:
        for blk in fn.blocks:
            insts = list(blk.instructions)
            out = []
            for inst in insts:
                si = inst.sync_info
                waits = list(si.on_wait) if (si and si.on_wait) else []
                if len(waits) > MAXW:
                    for i in range(0, len(waits) - MAXW, MAXW):
                        nop = mybir.InstNoOp(
                            name=f"WSPLIT-{nsplit}", ins=[], outs=[]
                        )
                        nsplit += 1
                        nop.engine = inst.engine
                        nop.sync_info = mybir.SyncInfo(
                            on_wait=waits[i : i + MAXW], on_update=[]
                        )
                        out.append(nop)
                    si.on_wait = waits[len(waits) - MAXW :]
                out.append(inst)
            if len(out) != len(insts):
                while len(blk.instructions):
                    blk.instructions.pop()
                for inst in out:
                    blk.instructions.append(inst)
    return nsplit


def build_nc(H=256, split_waits=True):
    _install_tile_patch()
    nc = bass.Bass()
    emit(nc, H)
    if split_waits:
        n = _split_multi_waits(nc)
        if n:
            print(f"kernel: split {n} multi-wait instructions")
    return nc


# ---------------------------------------------------------------------------
# host-side input prep
# ---------------------------------------------------------------------------
def prep_core_inputs(x_img, ids_img, w0, g0v, b0v, w1, g1v, b1v, H=256):
    HH = H // 2
    NPC = HH * 2
    HW2 = HH * W
    seg = np.where(ids_img < 0, 8, ids_img).astype(np.int64)
    segA, segB = seg[:HH], seg[HH:]

    m = {}
    xpad = np.zeros((C, (H + 1) * PITCH), np.float16)
    xr = xpad.reshape(C, H + 1, PITCH)
    xr[:, :H, LP : LP + W] = x_img.astype(np.float16)
    m["xh"] = np.ascontiguousarray(xpad)
    cnt = np.bincount(seg.reshape(-1), minlength=9)[:9]
    rc9 = (1.0 / np.maximum(cnt, 1)).astype(np.float32)
    
    for name, wmat in (("w0d", w0), ("w1d", w1)):
        wd = np.zeros((9, 128, 128), np.float16)
        for t in range(9):
            dy, dx = t // 3, t % 3
            lhsT = wmat[:, :, dy, dx].T.astype(np.float16)  # [cin, cout]
            wd[t, 0:64, 0:64] = lhsT
            wd[t, 64:128, 64:128] = lhsT
        m[name] = np.ascontiguousarray(wd.transpose(1, 0, 2))  # [ci, t, co]

    m["id128"] = np.eye(128, dtype=np.float16)

    # pixel-major ids [128, 2, NPC]: [:,0,ci]=A-seg, [:,1,ci]=B-seg of the
    # pixel (r, cs*128+p), ci = 2*r + cs; stats mask built on-chip from this
    pA = segA.reshape(HH, 2, 128).transpose(2, 0, 1).reshape(128, NPC)
    pB = segB.reshape(HH, 2, 128).transpose(2, 0, 1).reshape(128, NPC)
    m["idp"] = np.ascontiguousarray(
        np.stack([pA, pB], 1).reshape(128, 2 * NPC).astype(np.float16))
    m["krow"] = np.ascontiguousarray(
        np.broadcast_to(np.arange(9, dtype=np.float16), (128, 9))).copy()

    # raw ids for on-chip expansion-mask build
    m["idsf"] = np.ascontiguousarray(
        np.stack([segA.reshape(-1), segB.reshape(-1)]).astype(np.float16))
    kv = np.full(41, np.nan, np.float32)
    kv[0:9] = np.arange(9)
    kv[32:41] = np.arange(9)
    m["kv41"] = kv

    P = np.zeros((41, 41), np.float32)
    for s in range(9):
        P[32 + s, s] = 1.0
        P[s, 32 + s] = 1.0
    m["perm"] = P

    rc41 = np.zeros((41,), np.float32)
    rc41[0:9] = rc9
    rc41[32:41] = rc9
    m["rc"] = rc41

    for name, v in (("g0", g0v), ("b0", b0v), ("g1", g1v), ("b1", b1v)):
        t = np.zeros((41, 64), np.float32)
        t[0:9] = np.asarray(v, np.float32)[None, :]
        t[32:41] = np.asarray(v, np.float32)[None, :]
        m[name] = t
    return m


LAST_RESULT = None


def kernel(features, ins_indices_batch, w0, g0, b0, w1, g1, b1):
    global LAST_RESULT
    _install_ntff_shim()
    from concourse.bass_utils import run_bass_kernel_spmd
    from concourse import bass2jax as _b2j
    import traceback as _tb

    _b2j.install_neuronx_cc_hook()
    import libneuronxla as _lnx

    if not getattr(_lnx, "_ant_dbg_wrapped", False):
        _orig = _lnx.neuronx_cc

        def _dbg(*a, **k):
            try:
                return _orig(*a, **k)
            except BaseException:
                _tb.print_exc()
                raise

        _lnx.neuronx_cc = _dbg
        _lnx._ant_dbg_wrapped = True

    x = np.asarray(features, np.float32)
    ids = np.asarray(ins_indices_batch).astype(np.int64)
    w0 = np.asarray(w0, np.float32)
    w1 = np.asarray(w1, np.float32)
    N = x.shape[0]
    H = x.shape[2]

    nc = build_nc(H)
    in_maps = [
        prep_core_inputs(x[i], ids[i], w0, g0, b0, w1, g1, b1, H) for i in range(N)
    ]
    trace = bool(int(os.environ.get("BASS_KERNEL_TRACE", "0")))
    res = run_bass_kernel_spmd(nc, in_maps, list(range(N)), trace=trace)
    LAST_RESULT = res
    outs = []
    for i in range(N):
        qs = [res.results[i][f"out{q}"].reshape(C, H // 4, W) for q in range(4)]
        outs.append(np.concatenate(qs, axis=1).astype(np.float32))
    return np.stack(outs, 0)


# revision 5
# speedup vs baseline: 1.0200x; 1.0030x over previous
"""Trainium2 Bass kernel v2 for nn_DensePoseV1ConvXGNInsHead:
2x (conv3x3 64->64 -> per-instance BN -> ReLU) on [8,64,256,256],
one image per NeuronCore across 8 cores.

v2 redesign vs baseline:
- conv matmuls use full 128-partition block-diagonal weights (one MM per
  tap for both row-halves) instead of 64x64 quadrant pairs
- stats finalize swaps partition halves with a PE permutation matmul
  (no serial SBUF->SBUF DMAs on the critical path)
- all masks host-prepared and DMA'd contiguously
- L0-normalize and L1-conv interleaved in emission order so PE never drains
- f16 output
"""
import os
import sys
import types

sys.path.insert(0, "/opt/trn_rl_repo")

import numpy as np

import concourse.bass as bass
import concourse.tile as tile
from concourse import mybir
from concourse.vector_clock import ScopedClock

f16 = mybir.dt.float16
f32 = mybir.dt.float32
ALU = mybir.AluOpType
ACTF = mybir.ActivationFunctionType

C = 64          # channels
W = 256         # image width
PITCH = 272     # padded row pitch (16 left pad + 256 data; borrows next row's pad)
LP = 16         # left pad elements
EPS = 1e-5

# ---------------------------------------------------------------------------
# walrus workaround: split the Tile exit-drain's sem waits (installed walrus
# rejects instructions with >2 sync waits)
# ---------------------------------------------------------------------------
_patched = False


def _install_tile_patch():
    global _patched
    if _patched:
        return
    _patched = True

    def _drain_and_barrier(self, tick_clock, wait_clock):
        nc = self.nc
        drain_inst = nc.sync.drain()
        wait_clock.add_sem_waits(
            drain_inst.ins, ScopedClock({None: tick_clock.global_clock})
        )
        si = drain_inst.ins.sync_info
        waits = list(si.on_wait or [])
        if len(waits) > 1:
            si.on_wait = waits[:1]
            for i in range(1, len(waits)):
                nop = nc.sync.nop()
                nop.ins.sync_info = mybir.SyncInfo(
                    on_wait=waits[i : i + 1], on_update=[]
                )
        nc.all_engine_barrier()
        popped = nc._tile_sem_poison_stack.pop()
        assert popped is self._sem_poison
        nc.clear_and_free_semaphores(list(self.sems.allocated().values()))
        nc.all_engine_barrier()

    tile.TileContext._drain_and_barrier = _drain_and_barrier


# ---------------------------------------------------------------------------
# NTFF profiling shim (antenv.axon_hooks is absent in this image)
# ---------------------------------------------------------------------------
def _install_ntff_shim():
    if "antenv.axon_hooks" in sys.modules:
        return
    mod = types.ModuleType("antenv.axon_hooks")
    state = {"hook": None}
    mod.set_axon_ntff_profile_hook = lambda h: state.__setitem__("hook", h)
    mod.get_axon_ntff_profile_hook = lambda: state["hook"]
    sys.modules["antenv.axon_hooks"] = mod
    try:
        import antenv

        antenv.axon_hooks = mod
    except ImportError:
        pass
    try:
        from trn_agent_boot.trn_boot import _ntff_profile_via_ctypes

        h = _ntff_profile_via_ctypes("/opt/axon/libaxon_pjrt.so")
        mod.set_axon_ntff_profile_hook(h)
    except Exception:
        pass


def yoff(slot):
    return slot * PITCH + LP


def _ap(base_ap, offset_elems, dims):
    """Sub-AP of base_ap at +offset (elements) with given free dims."""
    return bass.AP(
        tensor=base_ap.tensor,
        offset=base_ap.offset + offset_elems,
        ap=[base_ap.ap[0]] + dims,
    )


def emit(nc, H=256):
    HH = H // 2          # 128 rows per half
    NCH = HH // 2        # conv chunks (2 rows) per layer = 64
    NPC = NCH * 4        # pixel chunks (128 px) per layer = 256
    NB = HH // 4         # x-load blocks (4 rows) = 32
    HW2 = HH * W

    xh = nc.declare_dram_parameter("xh", [C, (H + 1) * PITCH], f16, isOutput=False)
    w0d = nc.declare_dram_parameter("w0d", [128, 9, 128], f16, isOutput=False)
    w1d = nc.declare_dram_parameter("w1d", [128, 9, 128], f16, isOutput=False)
    id128 = nc.declare_dram_parameter("id128", [128, 128], f16, isOutput=False)
    mpm_d = nc.declare_dram_parameter("mpm", [128, NPC * 41], f16, isOutput=False)
    idsf_d = nc.declare_dram_parameter("idsf", [2, HW2], f16, isOutput=False)
    kv41_d = nc.declare_dram_parameter("kv41", [41], f32, isOutput=False)
    perm_d = nc.declare_dram_parameter("perm", [41, 41], f32, isOutput=False)
    rc_d = nc.declare_dram_parameter("rc", [41], f32, isOutput=False)
    g0d = nc.declare_dram_parameter("g0", [41, 64], f32, isOutput=False)
    b0d = nc.declare_dram_parameter("b0", [41, 64], f32, isOutput=False)
    g1d = nc.declare_dram_parameter("g1", [41, 64], f32, isOutput=False)
    b1d = nc.declare_dram_parameter("b1", [41, 64], f32, isOutput=False)
    outq = [
        nc.declare_dram_parameter(f"out{q}", [C, (H // 4) * W], f16, isOutput=True)
        for q in range(4)
    ]

    with tile.TileContext(nc) as tc:
        import contextlib

        with contextlib.ExitStack() as ctx:
            const = ctx.enter_context(tc.tile_pool(name="const", bufs=1))
            xbp = ctx.enter_context(tc.tile_pool(name="xbp", bufs=1))
            spp = ctx.enter_context(tc.tile_pool(name="spp", bufs=5))
            normp = ctx.enter_context(tc.tile_pool(name="normp", bufs=3))
            outp = ctx.enter_context(tc.tile_pool(name="outp", bufs=4))
            smallp = ctx.enter_context(tc.tile_pool(name="smallp", bufs=2))
            psc = ctx.enter_context(tc.tile_pool(name="psc", bufs=2, space="PSUM"))
            ptp = ctx.enter_context(tc.tile_pool(name="ptp", bufs=2, space="PSUM"))
            pss = ctx.enter_context(tc.tile_pool(name="pss", bufs=1, space="PSUM"))
            pse = ctx.enter_context(tc.tile_pool(name="pse", bufs=2, space="PSUM"))
            psw = ctx.enter_context(tc.tile_pool(name="psw", bufs=1, space="PSUM"))

            # ---- persistent y buffer (both layers, in-place)
            ysb = const.tile([128, (HH + 2) * PITCH + LP], f16)
            # zero only what stays zero: inter-row pads + top/bottom halo slots
            nc.vector.memset(_ap(ysb[:], 0, [[PITCH, HH + 3], [1, LP]]), 0.0)
            nc.vector.memset(_ap(ysb[:], yoff(0), [[1, W]]), 0.0)
            nc.vector.memset(_ap(ysb[:], yoff(HH + 1), [[1, W]]), 0.0)
            xb0 = xbp.tile([128, 6 * PITCH + LP], f16, tag="xb0")
            xb1 = xbp.tile([128, 6 * PITCH + LP], f16, tag="xb1")
            xbs = [xb0, xb1]

            # ---- constants from DRAM; only weights + identity loaded NOW.
            # Masks and finalize constants are DMA'd later (emission order =
            # queue order: they must not delay the first x loads).
            wts = []
            for wd in (w0d, w1d):
                wt = const.tile([128, 9, 128], f16, tag="wt", name=f"wt{len(wts)}")
                nc.sync.dma_start(out=wt[:], in_=wd[:])
                wts.append(wt)
            id128sb = const.tile([128, 128], f16)
            nc.sync.dma_start(out=id128sb[:], in_=id128[:])
            # stats mask host-built, DMA'd in 16 slices alternating both DGE
            # queues (one big dma_start would serialize on a single engine);
            # slice k covers pixel chunks [16k, 16k+16) -- consumption order
            mpm = const.tile([128, NPC, 41], f16)
            MPS = NPC * 41 // 16

            def load_mpm_slice(k):
                eng = nc.sync if k % 2 == 0 else nc.scalar
                eng.dma_start(
                    out=_ap(mpm[:], k * MPS, [[1, MPS]]),
                    in_=bass.AP(tensor=mpm_d[:].tensor, offset=k * MPS,
                                ap=[[NPC * 41, 128], [1, MPS]]),
                )
            ms2sb = const.tile([41, HW2], f16)
            kv41 = const.tile([41, 1], f32)
            nc.scalar.dma_start(
                out=kv41[:], in_=kv41_d[:].rearrange("(a b) -> a b", b=1))
            MSL = HW2 // 16

            def gen_ms2_slice(k):
                """Fill ms2sb[:, k*MSL:(k+1)*MSL] from broadcast ids.

                Rows 0:9 <- (idA==s), rows 32:41 <- (idB==s); rows 9:32
                compare stale data against NaN -> always 0."""
                ids41 = spp.tile([41, MSL], f16, tag="ids41", name=f"ids41_{k}",
                                 bufs=2)
                nc.sync.dma_start(
                    out=ids41[0:9, :],
                    in_=bass.AP(tensor=idsf_d[:].tensor, offset=k * MSL,
                                ap=[[0, 9], [1, MSL]]),
                )
                nc.scalar.dma_start(
                    out=ids41[32:41, :],
                    in_=bass.AP(tensor=idsf_d[:].tensor, offset=HW2 + k * MSL,
                                ap=[[0, 9], [1, MSL]]),
                )
                nc.vector.tensor_scalar(
                    out=ms2sb[:, k * MSL : (k + 1) * MSL], in0=ids41[:],
                    scalar1=kv41[:], scalar2=None, op0=ALU.is_equal,
                )
            permsb = const.tile([41, 41], f32)
            rcsb = const.tile([41, 1], f32)
            gbt = []
            for li in (0, 1):
                gt = const.tile([41, 64], f32, tag="g18", name=f"g18_{li}")
                bt = const.tile([41, 64], f32, tag="b18", name=f"b18_{li}")
                gbt.append((gt, bt))
            epsap = const.tile([41, 1], f32)
            nc.vector.memset(epsap[:], EPS)

            def load_late_consts_2():
                # finalize constants (scalar DGE queue; needed ~180us in)
                nc.scalar.dma_start(out=permsb[:], in_=perm_d[:])
                nc.scalar.dma_start(
                    out=rcsb[:], in_=rc_d[:].rearrange("(a b) -> a b", b=1))
                for li, (gg, bb) in enumerate(((g0d, b0d), (g1d, b1d))):
                    nc.scalar.dma_start(out=gbt[li][0][:], in_=gg[:])
                    nc.scalar.dma_start(out=gbt[li][1][:], in_=bb[:])



            # ---- per-layer affine tables [18, 128] f16 (block structure):
            # rows 0:9 hold A-seg values in cols 0:64, rows 9:18 B-seg in 64:128.
            # Background rows 8 / 17: scale 1, offset 0 (set once, never touched).
            ones16 = const.tile([1, 64], f16)
            nc.vector.memset(ones16[:], 1.0)
            abts = []
            for L in (0, 1):
                s_t = const.tile([41, 128], f16, tag="abs", name=f"ab2s{L}")
                o_t = const.tile([41, 128], f16, tag="abo", name=f"ab2o{L}")
                nc.vector.memset(s_t[:], 0.0)
                nc.vector.memset(o_t[:], 0.0)
                # background rows (seg 8): scale 1, offset 0
                nc.scalar.dma_start(out=s_t[8:9, 0:64], in_=ones16[:])
                nc.scalar.dma_start(out=s_t[40:41, 64:128], in_=ones16[:])
                abts.append((s_t, o_t))

            # =============================================================
            # emission helpers
            # =============================================================
            def load_x_block(b):
                """Load x rows for 4-row block b into xb[b%2] (rows r0-1..r0+4).

                xh is PITCH-padded in DRAM, so each half loads as ONE fully
                contiguous run per channel (3.3KB packets, pads included)."""
                xb = xbs[b % 2]
                r0 = 4 * b
                XP = (H + 1) * PITCH

                def half_load(part_lo, part_hi, xb_off, row_lo, n_elems, coff):
                    eng = nc.sync if part_lo == 0 else nc.scalar
                    eng.dma_start(
                        out=_ap(xb[part_lo:part_hi, :], xb_off, [[1, n_elems]]),
                        in_=bass.AP(
                            tensor=xh[:].tensor,
                            offset=coff + row_lo * PITCH,
                            ap=[[XP, 64], [1, n_elems]],
                        ),
                    )

                if b == 0:
                    nc.vector.memset(xb[0:64, 0:PITCH], 0.0)
                    # A rows 0..4 -> slots 1..5 (incl. borrow tail)
                    half_load(0, 64, PITCH, 0, 5 * PITCH + LP, 0)
                    # B rows HH-1..HH+4 -> slots 0..5
                    half_load(64, 128, 0, HH - 1, 6 * PITCH + LP, 0)
                elif b == NB - 1:
                    # A rows r0-1..r0+4 (row r0+4 == HH exists)
                    half_load(0, 64, 0, r0 - 1, 6 * PITCH + LP, 0)
                    nc.vector.memset(
                        xb[64:128, 5 * PITCH : 6 * PITCH + LP], 0.0)
                    # B rows HH+r0-1..HH+r0+3 -> slots 0..4
                    half_load(64, 128, 0, HH + r0 - 1, 5 * PITCH + LP, 0)
                else:
                    half_load(0, 64, 0, r0 - 1, 6 * PITCH + LP, 0)
                    half_load(64, 128, 0, HH + r0 - 1, 6 * PITCH + LP, 0)

            def conv_chunk(L, c):
                """9-tap conv for 2-row chunk c (both halves) + copy to ysb."""
                wt = wts[L]
                slot0 = 1 if L == 0 else 0
                rr = 2 * c
                pts = psc.tile([128, 512], f32, tag="cps", name=f"cps_{L}_{c}")
                if L == 0:
                    src_t = xbs[(c // 2) % 2]
                    base_slot = 2 * (c % 2) + 1  # row rr at block slot
                else:
                    src_t = ysb
                    base_slot = rr + 1
                for t in range(9):
                    dy, dx = t // 3 - 1, t % 3 - 1
                    off = yoff(base_slot + dy) + dx
                    rhs = _ap(src_t[:], off, [[PITCH, 2], [1, W]])
                    nc.tensor.matmul(
                        pts[:], _ap(wt[:], t * 128, [[1, 128]]), rhs,
                        start=(t == 0), stop=(t == 8),
                    )
                dst = _ap(ysb[:], yoff(rr + slot0), [[PITCH, 2], [1, W]])
                nc.vector.tensor_copy(dst, pts[:])

            def stats_transpose(L, c):
                """PE-transpose the 4 128-px chunks of conv chunk c."""
                slot0 = 1 if L == 0 else 0
                pts2 = ptp.tile([128, 512], f16, tag="tp", name=f"tp_{L}_{c}")
                for j in range(4):
                    src = _ap(
                        ysb[:], yoff(2 * c + j // 2 + slot0) + (j % 2) * 128,
                        [[1, 128]],
                    )
                    nc.tensor.transpose(
                        pts2[:, j * 128 : (j + 1) * 128], src, id128sb[:]
                    )
                return pts2

            def stats_strip(L, c, pts2):
                """Copy transposed y to SBUF and square it."""
                sp = spp.tile([128, 1024], f16, tag="strip", name=f"sp_{L}_{c}")
                nc.scalar.copy(out=sp[:, 0:512], in_=pts2[:])
                nc.vector.tensor_tensor(
                    sp[:, 512:1024], sp[:, 0:512], sp[:, 0:512], ALU.mult
                )
                return sp

            def stats_mm(L, c, sp, stats):
                for j in range(4):
                    ci = 4 * c + j
                    nc.tensor.matmul(
                        stats[:],
                        _ap(mpm[:], ci * 41, [[1, 41]]),
                        _ap(sp[:], j * 128, [[512, 2], [1, 128]]),
                        start=(ci == 0), stop=(ci == NPC - 1),
                    )

            def finalize(L, stats):
                """stats psum [41,256] -> affine tables ab2s/ab2o for layer L.

                Rows 0:9 = A-half segment sums, rows 32:41 = B-half (32-aligned
                partition bases for engine-op legality); rows 9:32 stay zero."""
                gt, bt = gbt[L]
                s_t, o_t = abts[L]
                ssb = smallp.tile([41, 256], f32, tag="ssb", name=f"ssb{L}")
                nc.scalar.copy(out=ssb[:], in_=stats[:])
                # PE permutation: swp[s] <-> ssb[32+s] (A/B half swap)
                swp = psw.tile([41, 256], f32, tag="swp", name=f"swp{L}")
                nc.tensor.matmul(swp[:], permsb[:], ssb[:], start=True, stop=True)
                s1 = smallp.tile([41, 64], f32, tag="s1", name=f"s1_{L}")
                s2 = smallp.tile([41, 64], f32, tag="s2", name=f"s2_{L}")
                nc.vector.memset(s1[:], 0.0)
                nc.vector.memset(s2[:], 0.0)
                # own half data in own cols; other half arrives swapped
                nc.vector.tensor_tensor(
                    s1[0:9, :], ssb[0:9, 0:64], swp[0:9, 64:128], ALU.add)
                nc.vector.tensor_tensor(
                    s1[32:41, :], ssb[32:41, 64:128], swp[32:41, 0:64], ALU.add)
                nc.vector.tensor_tensor(
                    s2[0:9, :], ssb[0:9, 128:192], swp[0:9, 192:256], ALU.add)
                nc.vector.tensor_tensor(
                    s2[32:41, :], ssb[32:41, 192:256], swp[32:41, 128:192], ALU.add)
                mean = smallp.tile([41, 64], f32, tag="mean", name=f"mean{L}")
                nc.vector.tensor_scalar_mul(out=mean[:], in0=s1[:], scalar1=rcsb[:])
                e2 = smallp.tile([41, 64], f32, tag="e2", name=f"e2_{L}")
                nc.vector.tensor_scalar_mul(out=e2[:], in0=s2[:], scalar1=rcsb[:])
                var = smallp.tile([41, 64], f32, tag="var", name=f"var{L}")
                nc.vector.tensor_tensor(var[:], mean[:], mean[:], ALU.mult)
                nc.vector.tensor_tensor(var[:], e2[:], var[:], ALU.subtract)
                sd = smallp.tile([41, 64], f32, tag="sd", name=f"sd{L}")
                nc.scalar.activation(
                    out=sd[:], in_=var[:], func=ACTF.Sqrt, bias=epsap[:], scale=1.0
                )
                rstd = smallp.tile([41, 64], f32, tag="rstd", name=f"rstd{L}")
                nc.vector.reciprocal(out=rstd[:], in_=sd[:])
                a = smallp.tile([41, 64], f32, tag="a", name=f"a{L}")
                nc.vector.tensor_tensor(a[:], rstd[:], gt[:], ALU.mult)
                am = smallp.tile([41, 64], f32, tag="am", name=f"am{L}")
                nc.vector.tensor_tensor(am[:], a[:], mean[:], ALU.mult)
                bv = smallp.tile([41, 64], f32, tag="bv", name=f"bv{L}")
                nc.vector.tensor_tensor(bv[:], bt[:], am[:], ALU.subtract)
                # quadrant placement (aligned bases; skip bg rows 8/40)
                nc.vector.tensor_copy(s_t[0:8, 0:64], a[0:8, :])
                nc.scalar.copy(out=s_t[32:40, 64:128], in_=a[32:40, :])
                nc.vector.tensor_copy(o_t[0:8, 0:64], bv[0:8, :])
                nc.scalar.copy(out=o_t[32:40, 64:128], in_=bv[32:40, :])

            def norm_group(L, g):
                """Normalize+ReLU rows 2g,2g+1 (both halves) of layer L.

                L=1 (tail phase, no conv to overlap): sE copy goes to DVE and
                oE lives in the psc pool (idle in phase C) for deeper overlap."""
                s_t, o_t = abts[L]
                slot0 = 1 if L == 0 else 0
                win = ms2sb[:, g * 512 : (g + 1) * 512]
                sE = pse.tile([128, 512], f32, tag="exp", name=f"se{L}_{g}")
                if L == 0:
                    oE = pse.tile([128, 512], f32, tag="exp", name=f"oe{L}_{g}")
                else:
                    oE = psc.tile([128, 512], f32, tag="cps", name=f"oe{L}_{g}")
                nc.tensor.matmul(sE[:], s_t[:], win, start=True, stop=True)
                nc.tensor.matmul(oE[:], o_t[:], win, start=True, stop=True)
                sEc = normp.tile([128, 512], f16, tag="sEc", name=f"sec{L}_{g}")
                oEc = normp.tile([128, 512], f16, tag="oEc", name=f"oec{L}_{g}")
                if L == 0:
                    nc.scalar.copy(out=sEc[:], in_=sE[:])
                else:
                    nc.vector.tensor_copy(sEc[:], sE[:])
                nc.scalar.copy(out=oEc[:], in_=oE[:])
                yv = _ap(ysb[:], yoff(2 * g + slot0), [[PITCH, 2], [1, W]])
                t1 = normp.tile([128, 512], f16, tag="t1", name=f"t1_{L}_{g}")
                u = normp.tile([128, 512], f16, tag="u", name=f"u_{L}_{g}")
                nc.vector.tensor_tensor(t1[:], yv, sEc[:], ALU.mult)
                nc.vector.tensor_tensor(u[:], t1[:], oEc[:], ALU.add)
                if L == 1:
                    return u
                if g % 2 == 0:
                    nc.vector.tensor_scalar_max(out=yv, in0=u[:], scalar1=0.0)
                else:
                    nc.scalar.activation(out=yv, in_=u[:], func=ACTF.Relu)
                return None

            def relu_store(g, u):
                """Deferred ReLU + store for phase-C group g (runs one group
                behind the mult/add so ACT never waits on the current DVE)."""
                st = outp.tile([128, 512], f16, tag="st", name=f"st{g}")
                nc.scalar.activation(out=st[:], in_=u[:], func=ACTF.Relu)
                # quarter q holds image rows [q*64, q*64+64); A-half rows
                # 2g,2g+1 land in quarter g//32, B-half in 2 + g//32
                qa = g // 32
                go = g % 32
                eng = nc.sync if g % 2 == 0 else nc.scalar
                HQ = (H // 4) * W
                eng.dma_start(
                    out=bass.AP(
                        tensor=outq[qa][:].tensor, offset=2 * go * W,
                        ap=[[HQ, 64], [W, 2], [1, W]],
                    ),
                    in_=st[0:64, :],
                )
                eng.dma_start(
                    out=bass.AP(
                        tensor=outq[2 + qa][:].tensor, offset=2 * go * W,
                        ap=[[HQ, 64], [W, 2], [1, W]],
                    ),
                    in_=st[64:128, :],
                )

            # =============================================================
            # phase A: layer-0 conv + stats (software-pipelined)
            # =============================================================
            stats0 = pss.tile([41, 256], f32, tag="stats", name="stats0")
            load_x_block(0)
            load_x_block(1)
            for k in range(8):
                load_mpm_slice(k)
            pend = {}
            for c in range(NCH):
                conv_chunk(0, c)
                if c in (0, 1):
                    for k in range(8 + 4 * c, 12 + 4 * c):
                        load_mpm_slice(k)
                if c == 2:
                    load_late_consts_2()
                if 6 <= c < 38 and c % 2 == 0:
                    gen_ms2_slice((c - 6) // 2)
                if c % 2 == 1 and c // 2 + 2 < NB:
                    # prefetch after block c//2's last consumer (WAR via emission order)
                    load_x_block(c // 2 + 2)
                if c >= 1:
                    p2 = stats_transpose(0, c - 1)
                    pend[c - 1] = stats_strip(0, c - 1, p2)
                if c >= 5:
                    stats_mm(0, c - 5, pend.pop(c - 5), stats0)
            p2 = stats_transpose(0, NCH - 1)
            pend[NCH - 1] = stats_strip(0, NCH - 1, p2)
            for cc in range(NCH - 5, NCH):
                stats_mm(0, cc, pend.pop(cc), stats0)

            finalize(0, stats0)

            # =============================================================
            # phase B: layer-0 normalize interleaved with layer-1 conv+stats
            # norm group order: [63, 0, 1, ..., 62] so halos are ready early
            # =============================================================
            stats1 = pss.tile([41, 256], f32, tag="stats", name="stats1")
            G = [NCH - 1] + list(range(NCH - 1))
            LAG = 4
            for i in range(NCH + LAG):
                if i < NCH:
                    norm_group(0, G[i])
                if i == 0:
                    # B-half bottom: B slot 0 <- A row HH-1 (slot HH)
                    nc.sync.dma_start(
                        out=_ap(ysb[64:128, :], yoff(0), [[1, W]]),
                        in_=_ap(ysb[0:64, :], yoff(HH), [[1, W]]),
                    )
                if i == 1:
                    # A-half top halo: A slot HH+1 <- B row 0 (slot 1)
                    nc.sync.dma_start(
                        out=_ap(ysb[0:64, :], yoff(HH + 1), [[1, W]]),
                        in_=_ap(ysb[64:128, :], yoff(1), [[1, W]]),
                    )
                if i >= LAG:
                    c = i - LAG
                    conv_chunk(1, c)
                    if c >= 1:
                        p2 = stats_transpose(1, c - 1)
                        pend[c - 1] = stats_strip(1, c - 1, p2)
                    if c >= 5:
                        stats_mm(1, c - 5, pend.pop(c - 5), stats1)
            p2 = stats_transpose(1, NCH - 1)
            pend[NCH - 1] = stats_strip(1, NCH - 1, p2)
            for cc in range(NCH - 5, NCH):
                stats_mm(1, cc, pend.pop(cc), stats1)

            finalize(1, stats1)

            # =============================================================
            # phase C: layer-1 normalize -> out
            # =============================================================
            prev = None
            for g in range(NCH):
                u = norm_group(1, g)
                if prev is not None:
                    relu_store(prev[0], prev[1])
                prev = (g, u)
            relu_store(prev[0], prev[1])

    return nc


MAXW = 1


def _split_multi_waits(nc):
    """The installed walrus rejects instructions with >MAXW sync waits; hoist
    excess waits onto preceding same-engine nops."""
    nsplit = 0
    for fn in nc.m.functions:
        for blk in fn.blocks:
            insts = list(blk.instructions)
            out = []
            for inst in insts:
                si = inst.sync_info
                waits = list(si.on_wait) if (si and si.on_wait) else []
                if len(waits) > MAXW:
                    for i in range(0, len(waits) - MAXW, MAXW):
                        nop = mybir.InstNoOp(
                            name=f"WSPLIT-{nsplit}", ins=[], outs=[]
                        )
                        nsplit += 1
                        nop.engine = inst.engine
                        nop.sync_info = mybir.SyncInfo(
                            on_wait=waits[i : i + MAXW], on_update=[]
                        )
                        out.append(nop)
                    si.on_wait = waits[len(waits) - MAXW :]
                out.append(inst)
            if len(out) != len(insts):
                while len(blk.instructions):
                    blk.instructions.pop()
                for inst in out:
                    blk.instructions.append(inst)
    return nsplit


def build_nc(H=256, split_waits=True):
    _install_tile_patch()
    nc = bass.Bass()
    emit(nc, H)
    if split_waits:
        n = _split_multi_waits(nc)
        if n:
            print(f"kernel: split {n} multi-wait instructions")
    return nc


# ---------------------------------------------------------------------------
# host-side input prep
# ---------------------------------------------------------------------------
def prep_core_inputs(x_img, ids_img, w0, g0v, b0v, w1, g1v, b1v, H=256):
    HH = H // 2
    NPC = HH * 2
    HW2 = HH * W
    seg = np.where(ids_img < 0, 8, ids_img).astype(np.int64)
    segA, segB = seg[:HH], seg[HH:]

    m = {}
    xpad = np.zeros((C, (H + 1) * PITCH), np.float16)
    xr = xpad.reshape(C, H + 1, PITCH)
    xr[:, :H, LP : LP + W] = x_img.astype(np.float16)
    m["xh"] = np.ascontiguousarray(xpad)
    cnt = np.bincount(seg.reshape(-1), minlength=9)[:9]
    rc9 = (1.0 / np.maximum(cnt, 1)).astype(np.float32)
    
    for name, wmat in (("w0d", w0), ("w1d", w1)):
        wd = np.zeros((9, 128, 128), np.float16)
        for t in range(9):
            dy, dx = t // 3, t % 3
            lhsT = wmat[:, :, dy, dx].T.astype(np.float16)  # [cin, cout]
            wd[t, 0:64, 0:64] = lhsT
            wd[t, 64:128, 64:128] = lhsT
        m[name] = np.ascontiguousarray(wd.transpose(1, 0, 2))  # [ci, t, co]

    m["id128"] = np.eye(128, dtype=np.float16)

    # pixel-major stats mask [128, NPC, 41]: chunk ci = 2*r + cs covers
    # pixels (r, cs*128 + p); cols 0:9 A-seg one-hot, 32:41 B-seg one-hot
    pA = segA.reshape(HH, 2, 128).transpose(2, 0, 1).reshape(128, NPC)
    pB = segB.reshape(HH, 2, 128).transpose(2, 0, 1).reshape(128, NPC)
    ks9 = np.arange(9)
    pm = np.zeros((128, NPC, 41), np.float16)
    pm[:, :, 0:9] = pA[:, :, None] == ks9
    pm[:, :, 32:41] = pB[:, :, None] == ks9
    m["mpm"] = np.ascontiguousarray(pm.reshape(128, NPC * 41))

    # raw ids for on-chip expansion-mask build
    m["idsf"] = np.ascontiguousarray(
        np.stack([segA.reshape(-1), segB.reshape(-1)]).astype(np.float16))
    kv = np.full(41, np.nan, np.float32)
    kv[0:9] = np.arange(9)
    kv[32:41] = np.arange(9)
    m["kv41"] = kv

    P = np.zeros((41, 41), np.float32)
    for s in range(9):
        P[32 + s, s] = 1.0
        P[s, 32 + s] = 1.0
    m["perm"] = P

    rc41 = np.zeros((41,), np.float32)
    rc41[0:9] = rc9
    rc41[32:41] = rc9
    m["rc"] = rc41

    for name, v in (("g0", g0v), ("b0", b0v), ("g1", g1v), ("b1", b1v)):
        t = np.zeros((41, 64), np.float32)
        t[0:9] = np.asarray(v, np.float32)[None, :]
        t[32:41] = np.asarray(v, np.float32)[None, :]
        m[name] = t
    return m


LAST_RESULT = None


def kernel(features, ins_indices_batch, w0, g0, b0, w1, g1, b1):
    global LAST_RESULT
    _install_ntff_shim()
    from concourse.bass_utils import run_bass_kernel_spmd
    from concourse import bass2jax as _b2j
    import traceback as _tb

    _b2j.install_neuronx_cc_hook()
    import libneuronxla as _lnx

    if not getattr(_lnx, "_ant_dbg_wrapped", False):
        _orig = _lnx.neuronx_cc

        def _dbg(*a, **k):
            try:
                return _orig(*a, **k)
            except BaseException:
                _tb.print_exc()
                raise

        _lnx.neuronx_cc = _dbg
        _lnx._ant_dbg_wrapped = True

    x = np.asarray(features, np.float32)
    ids = np.asarray(ins_indices_batch).astype(np.int64)
    w0 = np.asarray(w0, np.float32)
    w1 = np.asarray(w1, np.float32)
    N = x.shape[0]
    H = x.shape[2]

    nc = build_nc(H)
    in_maps = [
        prep_core_inputs(x[i], ids[i], w0, g0, b0, w1, g1, b1, H) for i in range(N)
    ]
    trace = bool(int(os.environ.get("BASS_KERNEL_TRACE", "0")))
    res = run_bass_kernel_spmd(nc, in_maps, list(range(N)), trace=trace)
    LAST_RESULT = res
    outs = []
    for i in range(N):
        qs = [res.results[i][f"out{q}"].reshape(C, H // 4, W) for q in range(4)]
        outs.append(np.concatenate(qs, axis=1).astype(np.float32))
    return np.stack(outs, 0)
